# revision 39
# baseline (speedup 1.0000x reference)
"""Trainium2 Bass kernel for the caption-generation module (2-layer GRU
encoder-decoder + vocab projection + log_softmax).

Strategy: data-parallel over batch across 8 NeuronCores (B=128 -> 16 rows
per core, weights replicated).  Per core, transposed layout (feature dim on
SBUF partitions, (time*batch) on the free dim):

  E1:  gi1[t] = x_t @ w_ih1.T for all 40 encoder steps  (one batched matmul)
  C1/C2: h1/h2 chains, 67 sequential steps each, software-pipelined so the
       two chains' gate math interleaves op-by-op on Vector/Scalar while
       the PE runs the other chain's recurrent matmul.  gi (+b) for the
       r/z gates is preloaded into PSUM so the matmuls accumulate onto it
       and sigmoid reads PSUM directly.
  E3:  gi2[t] = [h1_t; w_t] @ w_ih2.T in blocks (batched matmul)
  P :  logits = h2_dec @ out_w.T (fp8), streamed log_softmax in two
       row-tile groups so the first group's output DMA overlaps the
       second group's projection; output written as bf16.
"""

import sys

sys.path.insert(0, "/opt/trn_rl_repo")

import numpy as np
import ml_dtypes

import concourse.bass as bass
import concourse.mybir as mybir
import concourse.tile as tile
from concourse.alu_op_type import AluOpType
from concourse.vector_clock import ScopedClock

# Align the Tile scheduler's PE cost model with measured hardware: a 16-col
# chain matmul sustains ~28ns/instruction on this device (issue-floor bound)
# vs ~7ns modeled at the 2.4GHz peak clock.  The skewed model makes the
# static scheduler bunch matmuls ahead of the gate ops and misorder the
# Scalar stream.  These constants are read lazily (once per process) by the
# rust cost model, so patch before the first build.  Schedule-order only —
# no effect on emitted semantics.
import concourse.hw_specs as _hw_specs

_hw_specs.TRN2Spec.PE_CYCLE = 1e9 / 0.6e9
_hw_specs.TRN2Spec.PE_CYCLE_PSTATE_MID = 1e9 / 0.55e9
_hw_specs.TRN2Spec.PE_CYCLE_PSTATE_LOW = 1e9 / 0.5e9

BF16 = mybir.dt.bfloat16
F32 = mybir.dt.float32
F8 = mybir.dt.float8e3
PSCALE = 64.0   # fp8 out_w pre-scale
HSCALE = 8.0    # fp8 h2 pre-scale
AF = mybir.ActivationFunctionType


# ---------------------------------------------------------------------------
# Workaround: this container's walrus rejects CTRL instructions carrying more
# than one sync-wait command.  Split the TileContext tail drain's wait list
# across a chain of drains, one wait each.
# ---------------------------------------------------------------------------
def _patched_drain_and_barrier(self, tick_clock, wait_clock):
    import bass_rust

    drain_inst = self.nc.sync.drain()
    wait_clock.add_sem_waits(
        drain_inst.ins, ScopedClock({None: tick_clock.global_clock})
    )
    waits = list(drain_inst.ins.sync_info.on_wait)
    if len(waits) > 1:
        si = drain_inst.ins.sync_info
        si.on_wait = waits[:1]
        drain_inst.ins.sync_info = si
        for i in range(1, len(waits)):
            extra = self.nc.sync.drain()
            extra.ins.sync_info = bass_rust.SyncInfo(
                on_wait=waits[i : i + 1], on_update=[]
            )
    self.nc.all_engine_barrier()
    assert self.sems is not None
    popped = self.nc._tile_sem_poison_stack.pop()
    assert popped is self._sem_poison
    self.nc.clear_and_free_semaphores(list(self.sems.allocated().values()))
    self.nc.all_engine_barrier()


tile.TileContext._drain_and_barrier = _patched_drain_and_barrier

# Same walrus limitation for regular engine instructions: at most one
# sync-wait per instruction.  Split extra waits onto preceding NoOps on the
# same engine (engine stalls there instead — identical semantics).
_orig_commit = tile.TileContext._commit_instruction


def _commit_split_waits(self, inst, lazy_reg_writes=True):
    si = getattr(inst, "sync_info", None)
    if (si is not None and si.on_wait and len(si.on_wait) > 1
            and inst.engine != mybir.EngineType.Unassigned):
        waits = list(si.on_wait)
        for w in waits[:-1]:
            nop = mybir.InstNoOp(
                name=self.nc.get_next_instruction_name(),
                sync_info=mybir.SyncInfo(on_wait=[w], on_update=[]),
                bass_nofuse=True,
                engine=inst.engine,
            )
            _orig_commit(self, nop, lazy_reg_writes=False)
        si.on_wait = waits[-1:]
        inst.sync_info = si
    return _orig_commit(self, inst, lazy_reg_writes)


tile.TileContext._commit_instruction = _commit_split_waits


# ---------------------------------------------------------------------------
# Config
# ---------------------------------------------------------------------------
def make_cfg(B=128, NF=40, TD=27, V=16000, DV=2048, DH=512, DW=512,
             n_cores=8, has_out_b=False, has_bias1=False, has_bhn1=False,
             has_bhn2=False):
    cfg = dict(B=B, NF=NF, TD=TD, V=V, DV=DV, DH=DH, DW=DW,
               n_cores=n_cores, has_out_b=has_out_b, has_bias1=has_bias1,
               has_bhn1=has_bhn1, has_bhn2=has_bhn2)
    cfg["BS"] = B // n_cores          # batch rows per core
    cfg["KV"] = DV // 128             # x feature chunks
    cfg["KH"] = DH // 128             # h feature chunks
    cfg["KW"] = DW // 128             # word feature chunks
    cfg["MC"] = 3 * DH // 128         # gate chunks
    cfg["NSTEP"] = NF + TD            # total chain steps
    cfg["ROWS_E"] = NF * cfg["BS"]    # encoder (t,b) columns
    cfg["ROWS_A"] = cfg["NSTEP"] * cfg["BS"]
    cfg["ROWS_D"] = TD * cfg["BS"]    # decode (t,b) columns
    # vocab tiling for the projection (psum free dim <= 512 fp32)
    for pn in (512, 500, 400, 320, 256):
        if V % pn == 0:
            cfg["PN"] = pn
            break
    else:
        raise ValueError(f"V={V} has no tile size")
    return cfg


def _ntiles(total, maxn):
    """Split `total` into tiles of at most maxn (last ragged)."""
    out = []
    n0 = 0
    while n0 < total:
        nn = min(maxn, total - n0)
        out.append((n0, nn))
        n0 += nn
    return out


# ---------------------------------------------------------------------------
# Kernel builder
# ---------------------------------------------------------------------------
def build_nc(cfg):
    BS, KV, KH, KW, MC = cfg["BS"], cfg["KV"], cfg["KH"], cfg["KW"], cfg["MC"]
    NF, TD, V, DH = cfg["NF"], cfg["TD"], cfg["V"], cfg["DH"]
    NSTEP, ROWS_E, ROWS_D = cfg["NSTEP"], cfg["ROWS_E"], cfg["ROWS_D"]
    PN = cfg["PN"]
    G3 = 3 * DH
    RZ = 2 * KH   # number of r+z gate chunks (psum-preloaded with gi)
    LAG = 12      # h2 chain trails h1 by this many steps (> E3 block size)

    nc = bass.Bass()

    # ---- DRAM parameters (per-core views; host prepares these) ----
    xT = nc.dram_tensor("xT", [cfg["DV"], ROWS_E], BF16, kind="ExternalInput")
    wordsT = nc.dram_tensor("wordsT", [cfg["DW"], ROWS_D], BF16, kind="ExternalInput")
    w1T = nc.dram_tensor("w1T", [cfg["DV"], G3], BF16, kind="ExternalInput")
    wh1T = nc.dram_tensor("wh1T", [DH, G3], BF16, kind="ExternalInput")
    w2T = nc.dram_tensor("w2T", [DH + cfg["DW"], G3], BF16, kind="ExternalInput")
    wh2T = nc.dram_tensor("wh2T", [DH, G3], BF16, kind="ExternalInput")
    owT = nc.dram_tensor("owT", [DH, V], F8, kind="ExternalInput")
    bi1c = nc.dram_tensor("bi1c", [128, MC], F32, kind="ExternalInput")
    bi2c = nc.dram_tensor("bi2c", [128, MC], F32, kind="ExternalInput")
    gidec = nc.dram_tensor("gidec", [128, MC, BS], BF16, kind="ExternalInput")
    ident = nc.dram_tensor("ident", [128, 128], BF16, kind="ExternalInput")
    bhnv1 = nc.dram_tensor("bhnv1", [1, DH], BF16, kind="ExternalInput")
    bhnv2 = nc.dram_tensor("bhnv2", [1, DH], BF16, kind="ExternalInput")
    onesb = nc.dram_tensor("onesb", [1, BS], BF16, kind="ExternalInput")
    if cfg["has_out_b"]:
        outb = nc.dram_tensor("outb", [1, V], F8, kind="ExternalInput")
        ones = nc.dram_tensor("ones", [1, 128], F8, kind="ExternalInput")
    out = nc.dram_tensor("out", [BS, TD, V], BF16, kind="ExternalOutput")
    # view [t, b, v] of out[b, t, v] (strides V, TD*V, 1); row r = t*BS + b
    _o = out[:]
    out_tbv = bass.AP(tensor=_o.tensor, offset=_o.offset,
                      ap=[[V, TD], [TD * V, BS], [1, V]])

    def out_slice(r0, mrows, c0, cw):
        assert r0 % BS == 0 and mrows % BS == 0
        return out_tbv[r0 // BS:(r0 + mrows) // BS, :, c0:c0 + cw]

    from contextlib import ExitStack

    with tile.TileContext(nc) as tc:
        with (
            tc.tile_pool(name="pconst", bufs=1) as pconst,
            tc.tile_pool(name="pchain", bufs=8) as pchain,
            tc.tile_pool(name="psum_mm", bufs=3, space="PSUM") as psum_mm,
            tc.tile_pool(name="psum_gh", bufs=5, space="PSUM") as psum_gh,
            tc.tile_pool(name="ph2", bufs=1) as ph2,
        ):
            # ---- constants ----
            bi1c_sb = pconst.tile([128, MC], F32, tag="bi1c")
            nc.sync.dma_start(out=bi1c_sb[:], in_=bi1c[:])
            bi2c_sb = pconst.tile([128, MC], F32, tag="bi2c")
            nc.sync.dma_start(out=bi2c_sb[:], in_=bi2c[:])
            gidec_sb = pconst.tile([128, MC, BS], BF16, tag="gidec")
            nc.sync.dma_start(out=gidec_sb[:], in_=gidec[:])
            ident_sb = pconst.tile([128, 128], BF16, tag="ident")
            nc.sync.dma_start(out=ident_sb[:], in_=ident[:])
            bhnv1_sb = pconst.tile([1, DH], BF16, tag="bhnv1")
            nc.sync.dma_start(out=bhnv1_sb[:], in_=bhnv1[:])
            bhnv2_sb = pconst.tile([1, DH], BF16, tag="bhnv2")
            nc.sync.dma_start(out=bhnv2_sb[:], in_=bhnv2[:])
            onesb_sb = pconst.tile([1, BS], BF16, tag="onesb")
            nc.sync.dma_start(out=onesb_sb[:], in_=onesb[:])
            if cfg["has_out_b"]:
                outb_sb = pconst.tile([1, V], F8, tag="outb")
                nc.sync.dma_start(out=outb_sb[:], in_=outb[:])
                ones_sb = pconst.tile([1, 128], F8, tag="ones")
                nc.sync.dma_start(out=ones_sb[:], in_=ones[:])

            h2_sb = ph2.tile([128, KH, (NSTEP + 1) * BS], BF16, tag="h2")
            nc.vector.memset(h2_sb[:, :, 0:BS], 0.0)

            # ====== E1, interleaved h1/E3/h2 chains ======
            with ExitStack() as chain_es:
                pmidA = chain_es.enter_context(tc.tile_pool(name="pmidA", bufs=1))
                h1_sb = pmidA.tile([128, KH, (NSTEP + 1) * BS], BF16, tag="h1")
                nc.vector.memset(h1_sb[:, :, 0:BS], 0.0)
                gi1_sb = pmidA.tile([128, MC, ROWS_E], BF16, tag="gi1")
                wh1_sb = pmidA.tile([128, KH, G3], BF16, tag="wh1")

                with tc.tile_pool(name="pw1", bufs=1) as pw1:
                    x_sb = pw1.tile([128, KV, ROWS_E], BF16, tag="x")
                    xT_r = xT[:].rearrange("(k p) n -> p k n", p=128)
                    for k in range(KV):
                        nc.sync.dma_start(out=x_sb[:, k, :], in_=xT_r[:, k, :])
                    w1_sb = pw1.tile([128, KV, G3], BF16, tag="w1")
                    w1T_r = w1T[:].rearrange("(k p) n -> p k n", p=128)
                    for k in range(KV):
                        nc.sync.dma_start(out=w1_sb[:, k, :], in_=w1T_r[:, k, :])
                    wh1T_r = wh1T[:].rearrange("(k p) n -> p k n", p=128)
                    for k in range(KH):
                        nc.sync.dma_start(out=wh1_sb[:, k, :], in_=wh1T_r[:, k, :])

                    # E1: gi1 = w1T.T @ x  (+ bias via ACT copy)
                    for (n0, nn) in _ntiles(ROWS_E, 320):
                        for m in range(MC):
                            ps = psum_mm.tile([128, 512], F32, tag="mm")
                            for k in range(KV):
                                nc.tensor.matmul(
                                    ps[:, :nn],
                                    lhsT=w1_sb[:, k, m * 128:(m + 1) * 128],
                                    rhs=x_sb[:, k, n0:n0 + nn],
                                    start=(k == 0), stop=(k == KV - 1))
                            nc.scalar.activation(
                                out=gi1_sb[:, m, n0:n0 + nn], in_=ps[:, :nn],
                                func=AF.Identity, bias=bi1c_sb[:, m:m + 1],
                                scale=1.0)

                # layer-2 weights / words / gi2 (loaded while chains run)
                pmidB = chain_es.enter_context(tc.tile_pool(name="pmidB", bufs=1))
                w2_sb = pmidB.tile([128, KH + KW, G3], BF16, tag="w2")
                w2T_r = w2T[:].rearrange("(k p) n -> p k n", p=128)
                for k in range(KH + KW):
                    nc.sync.dma_start(out=w2_sb[:, k, :], in_=w2T_r[:, k, :])
                words_sb = pmidB.tile([128, KW, ROWS_D], BF16, tag="words")
                wordsT_r = wordsT[:].rearrange("(k p) n -> p k n", p=128)
                for k in range(KW):
                    nc.sync.dma_start(out=words_sb[:, k, :], in_=wordsT_r[:, k, :])
                wh2_sb = pmidB.tile([128, KH, G3], BF16, tag="wh2")
                wh2T_r = wh2T[:].rearrange("(k p) n -> p k n", p=128)
                for k in range(KH):
                    nc.sync.dma_start(out=wh2_sb[:, k, :], in_=wh2T_r[:, k, :])
                gi2_sb = pmidB.tile([128, MC, NSTEP * BS], BF16, tag="gi2")

                # ---------------- chain machinery -------------------------
                # psum gh tile: one identity-matmul injects gi_rz (start=True
                # for the whole r/z region, so each m-region's accumulation
                # group closes at its own stop and consumers get fine-grained
                # waits: sigmoid fires right after the r-chunk matmuls).
                # m order: r chunks first, then n (np0), then z (hm1, last).
                M_ORDER = (list(range(KH)) + list(range(RZ, MC))
                           + list(range(KH, RZ)))
                gh_live = {}

                def girz_of(cid, t):
                    if cid == 1:
                        if t < NF:
                            return gi1_sb[:, 0:RZ, t * BS:(t + 1) * BS]
                        return gidec_sb[:, 0:RZ, :] if cfg["has_bias1"] else None
                    return gi2_sb[:, 0:RZ, t * BS:(t + 1) * BS]

                def gin_of(cid, t):
                    if cid == 1:
                        if t < NF:
                            return gi1_sb[:, RZ:, t * BS:(t + 1) * BS]
                        return gidec_sb[:, RZ:, :] if cfg["has_bias1"] else None
                    return gi2_sb[:, RZ:, t * BS:(t + 1) * BS]

                def chain_mms(cid, t):
                    gh = psum_gh.tile([128, MC, BS], F32, tag="gh")
                    gh_live[(cid, t)] = gh
                    whh = wh1_sb if cid == 1 else wh2_sb
                    hsb = h1_sb if cid == 1 else h2_sb
                    has_bhn = cfg["has_bhn1"] if cid == 1 else cfg["has_bhn2"]
                    bhnv_sb = bhnv1_sb if cid == 1 else bhnv2_sb
                    prev = hsb[:, :, t * BS:(t + 1) * BS]
                    inj = girz_of(cid, t)
                    if inj is not None:
                        nc.tensor.matmul(
                            gh[:, 0:RZ, :], lhsT=ident_sb[:], rhs=inj,
                            start=True, stop=False, skip_group_check=True)
                    for m in M_ORDER:
                        is_rz = m < RZ
                        if not is_rz and has_bhn:
                            # rank-1: gh_n += bhn_chunk (x) ones
                            nc.tensor.matmul(
                                gh[:, m, :],
                                lhsT=bhnv_sb[0:1, (m - RZ) * 128:(m - RZ + 1) * 128],
                                rhs=onesb_sb[0:1, :],
                                start=True, stop=False, skip_group_check=True)
                        for k in range(KH):
                            start = (k == 0) and (
                                (is_rz and inj is None)
                                or (not is_rz and not has_bhn))
                            nc.tensor.matmul(
                                gh[:, m, :],
                                lhsT=whh[:, k, m * 128:(m + 1) * 128],
                                rhs=prev[:, k, :],
                                start=start, stop=(k == KH - 1),
                                skip_group_check=True)

                def gate_sig_r(cid, t, st):
                    gh = gh_live[(cid, t)]
                    rzs = pchain.tile([128, RZ, BS], F32, tag=f"rzs{cid}")
                    nc.scalar.activation(out=rzs[:, 0:KH, :],
                                         in_=gh[:, 0:KH, :], func=AF.Sigmoid)
                    st["rzs"] = rzs

                def gate_sig_z(cid, t, st):
                    # emitted after tanh: z is needed only by hm1, so it must
                    # not sit between np0 and tanh in the Scalar stream
                    gh = gh_live[(cid, t)]
                    nc.scalar.activation(out=st["rzs"][:, KH:RZ, :],
                                         in_=gh[:, KH:RZ, :], func=AF.Sigmoid)

                def gate_np(cid, t, st):
                    # np0 reads the n-gate psum directly (includes bhn via the
                    # rank-1 matmul when nonzero); np1 adds gi_n when present.
                    # np0 reads PSUM so it stays on Vector for both chains;
                    # chain 2's remaining SBUF-only ops go to idle GpSimd.
                    gh = gh_live[(cid, t)]
                    eng1 = nc.vector
                    eng2 = nc.vector if cid == 1 else nc.gpsimd
                    np0 = pchain.tile([128, KH, BS], F32, tag=f"np0{cid}")
                    eng1.tensor_tensor(
                        out=np0[:], in0=st["rzs"][:, 0:KH, :],
                        in1=gh[:, RZ:, :], op=AluOpType.mult)
                    gin = gin_of(cid, t)
                    if gin is None:
                        st["np1"] = np0
                    else:
                        np1 = pchain.tile([128, KH, BS], F32, tag=f"np1{cid}")
                        eng2.tensor_tensor(
                            out=np1[:], in0=gin, in1=np0[:],
                            op=AluOpType.add)
                        st["np1"] = np1

                def gate_tanh(cid, st):
                    nt = pchain.tile([128, KH, BS], F32, tag=f"nt{cid}")
                    nc.scalar.activation(out=nt[:], in_=st["np1"][:],
                                         func=AF.Tanh)
                    st["nt"] = nt

                def gate_out(cid, t, st):
                    eng = nc.vector if cid == 1 else nc.gpsimd
                    hsb = h1_sb if cid == 1 else h2_sb
                    prev = hsb[:, :, t * BS:(t + 1) * BS]
                    hm0 = pchain.tile([128, KH, BS], F32, tag=f"hm0{cid}")
                    eng.tensor_tensor(
                        out=hm0[:], in0=prev[:], in1=st["nt"][:],
                        op=AluOpType.subtract)
                    hm1 = pchain.tile([128, KH, BS], F32, tag=f"hm1{cid}")
                    eng.tensor_tensor(
                        out=hm1[:], in0=st["rzs"][:, KH:RZ, :], in1=hm0[:],
                        op=AluOpType.mult)
                    eng.tensor_tensor(
                        out=hsb[:, :, (t + 1) * BS:(t + 2) * BS],
                        in0=st["nt"][:], in1=hm1[:], op=AluOpType.add)

                def e3_block(t0, nsteps):
                    """gi2 for chain steps [t0, t0+nsteps); drains split S/V."""
                    n0 = t0 * BS
                    nn = nsteps * BS
                    enc = t0 < NF  # blocks never straddle NF
                    for m in range(MC):
                        ps = psum_mm.tile([128, 512], F32, tag="mm")
                        for k in range(KH):
                            nc.tensor.matmul(
                                ps[:, :nn],
                                lhsT=w2_sb[:, k, m * 128:(m + 1) * 128],
                                rhs=h1_sb[:, k, BS + n0:BS + n0 + nn],
                                start=(k == 0),
                                stop=(enc and k == KH - 1))
                        if not enc:
                            w0 = n0 - ROWS_E
                            for k in range(KW):
                                nc.tensor.matmul(
                                    ps[:, :nn],
                                    lhsT=w2_sb[:, KH + k, m * 128:(m + 1) * 128],
                                    rhs=words_sb[:, k, w0:w0 + nn],
                                    start=False, stop=(k == KW - 1))
                        # drains on Vector only: Scalar carries the chains'
                        # sigmoid/tanh critical path
                        nc.vector.tensor_scalar_add(
                            gi2_sb[:, m, n0:n0 + nn], ps[:, :nn],
                            bi2c_sb[:, m:m + 1])

                blocks = ([(t0, nn) for (t0, nn) in _ntiles(NF, 8)] +
                          [(NF + t0, nn) for (t0, nn) in _ntiles(TD, 9)])
                block_end = {t0 + nn: (t0, nn) for (t0, nn) in blocks}

                # Emission order per pair is chosen so each engine's in-order
                # stream never blocks on the other chain's not-yet-ready op:
                #   PE: h1 mms, h2 mms, (e3)
                #   S : sigr1 sigz1 tanh1 sigr2 sigz2 tanh2
                #   V : preloads(next) tmpn1 np0_1 np1_1 tmpn2 hm_1* np_2* hm_2*
                st1, st2 = {}, {}
                for tt in range(NSTEP + LAG):
                    t1 = tt if tt < NSTEP else None
                    s2 = tt - LAG if 0 <= tt - LAG < NSTEP else None
                    if t1 is not None:
                        chain_mms(1, t1)
                    if s2 is not None:
                        chain_mms(2, s2)
                    if t1 is not None:
                        gate_sig_r(1, t1, st1)
                        gate_np(1, t1, st1)
                        gate_tanh(1, st1)
                        gate_sig_z(1, t1, st1)
                        gate_out(1, t1, st1)
                        del gh_live[(1, t1)]
                    if s2 is not None:
                        gate_sig_r(2, s2, st2)
                        gate_np(2, s2, st2)
                        gate_tanh(2, st2)
                        gate_sig_z(2, s2, st2)
                        gate_out(2, s2, st2)
                        del gh_live[(2, s2)]
                    if t1 is not None and t1 + 1 in block_end:
                        e3_block(*block_end[t1 + 1])

            # ---- projection + log_softmax over decode steps ----
            # |logits| is bounded well below fp32 exp overflow here, so
            # log_softmax runs without the max shift: lp = x - ln(sum(e^x)).
            # Two row-tile groups: group 0's lse/subtract/output-DMA overlap
            # group 1's projection.  out_w and h2 in fp8 (pre-scaled).
            with (
                tc.tile_pool(name="pp", bufs=1) as pp,
                tc.tile_pool(name="pwst", bufs=3) as pwst,
                tc.tile_pool(name="pstage", bufs=3) as pstage,
                tc.tile_pool(name="psmall", bufs=2) as psmall,
            ):
                owT_r = owT[:].rearrange("(k p) n -> p k n", p=128)
                nvt = V // PN
                dcol0 = (NF + 1) * BS  # first decode h2 col
                mtiles = _ntiles(ROWS_D, 128)
                inv_s = 1.0 / (PSCALE * HSCALE)
                # h2 decode block, fp8 pre-scaled by HSCALE
                h2f8 = pp.tile([128, KH, ROWS_D], F8, tag="h2f8")
                for k in range(KH):
                    nc.vector.tensor_scalar_mul(
                        h2f8[:, k, :], h2_sb[:, k, dcol0:dcol0 + ROWS_D],
                        HSCALE)
                logits_t = [pp.tile([128, V], BF16, tag=f"logits{i}",
                                    name=f"logits{i}")
                            for i in range(len(mtiles))]
                sums_t = [psmall.tile([128, nvt], F32, tag=f"sums{i}",
                                      name=f"sums{i}")
                          for i in range(len(mtiles))]
                NCH = 4
                CW = V // NCH
                for g0 in range(0, len(mtiles), 2):
                    grp = [(mt, mtiles[mt])
                           for mt in range(g0, min(g0 + 2, len(mtiles)))]
                    for nt_i in range(nvt):
                        n0 = nt_i * PN
                        wst = pwst.tile([128, KH, PN], F8, tag="wst")
                        nc.sync.dma_start(out=wst[:], in_=owT_r[:, :, n0:n0 + PN])
                        for mt, (r0, mrows) in grp:
                            ps = psum_mm.tile([128, 512], F32, tag="mm")
                            last = KH - 1 if not cfg["has_out_b"] else None
                            for k in range(KH):
                                nc.tensor.matmul(
                                    ps[:mrows, :PN],
                                    lhsT=h2f8[:, k, r0:r0 + mrows],
                                    rhs=wst[:, k, :],
                                    start=(k == 0), stop=(k == last))
                            if cfg["has_out_b"]:
                                nc.tensor.matmul(
                                    ps[:mrows, :PN],
                                    lhsT=ones_sb[:, :mrows],
                                    rhs=outb_sb[:, n0:n0 + PN],
                                    start=False, stop=True)
                            edump = pstage.tile([128, PN], BF16, tag="edump")
                            nc.scalar.activation(
                                out=edump[:mrows, :], in_=ps[:mrows, :PN],
                                func=AF.Exp, scale=inv_s,
                                accum_out=sums_t[mt][:mrows, nt_i:nt_i + 1])
                            nc.vector.tensor_scalar_mul(
                                logits_t[mt][:mrows, n0:n0 + PN],
                                ps[:mrows, :PN], inv_s)
                    # group tail: lse per row tile, then logp = logits - lse,
                    # alternating ACT / DVE per chunk; bf16 output via
                    # alternating gpsimd / sync DMA queues
                    for mt, (r0, mrows) in grp:
                        s1 = psmall.tile([128, 1], F32, tag="s1")
                        nc.vector.tensor_reduce(
                            out=s1[:mrows], in_=sums_t[mt][:mrows, :],
                            axis=mybir.AxisListType.X, op=AluOpType.add)
                        nshift = psmall.tile([128, 1], F32, tag="nshift")
                        nc.scalar.activation(
                            out=nshift[:mrows], in_=s1[:mrows], func=AF.Ln)
                        nc.vector.tensor_scalar_mul(
                            nshift[:mrows], nshift[:mrows], -1.0)
                        for c in range(NCH):
                            stage = pstage.tile([128, CW], BF16, tag="stage")
                            src = logits_t[mt][:mrows, c * CW:(c + 1) * CW]
                            if c % 2 == 0:
                                nc.scalar.activation(
                                    out=stage[:mrows, :], in_=src,
                                    func=AF.Identity, bias=nshift[:mrows])
                            else:
                                nc.vector.tensor_scalar_add(
                                    stage[:mrows, :], src, nshift[:mrows])
                            eng = nc.gpsimd if c % 2 == 0 else nc.sync
                            eng.dma_start(
                                out=out_slice(r0, mrows, c * CW, CW),
                                in_=stage[:mrows, :])
    return nc


# ---------------------------------------------------------------------------
# Host side
# ---------------------------------------------------------------------------
def _bf16(a):
    return np.ascontiguousarray(a, dtype=np.float32).astype(ml_dtypes.bfloat16)


def _f32(a):
    return np.ascontiguousarray(a, dtype=np.float32)


def _f8(a, scale):
    f8 = ml_dtypes.float8_e3m4
    f8max = float(ml_dtypes.finfo(f8).max)
    scaled = np.clip(np.asarray(a, dtype=np.float32) * scale, -f8max, f8max)
    return np.ascontiguousarray(scaled).astype(f8)


def prep_inputs(cfg, vid_feats, target_variable, emb, w_ih1, w_hh1, b_ih1,
                b_hh1, w_ih2, w_hh2, b_ih2, b_hh2, out_w, out_b):
    """Build per-core input maps."""
    BS, MC, KH, DH = cfg["BS"], cfg["MC"], cfg["KH"], cfg["DH"]
    TD, NC = cfg["TD"], cfg["n_cores"]

    vid_feats = np.asarray(vid_feats, dtype=np.float32)
    target_variable = np.asarray(target_variable)
    emb = np.asarray(emb, dtype=np.float32)

    shared = {
        "w1T": _bf16(np.asarray(w_ih1).T),
        "wh1T": _bf16(np.asarray(w_hh1).T),
        "w2T": _bf16(np.asarray(w_ih2).T),
        "wh2T": _bf16(np.asarray(w_hh2).T),
        "owT": _f8(np.asarray(out_w).T, PSCALE),
    }
    # combined biases: b_ih (+ b_hh for the r,z chunks; the n chunk of b_hh
    # is applied inside the gate, before the r multiply)
    def comb(bi, bh):
        c = np.asarray(bi, dtype=np.float32).copy()
        c[: 2 * DH] += np.asarray(bh, dtype=np.float32)[: 2 * DH]
        return c

    c1 = comb(b_ih1, b_hh1)
    c2 = comb(b_ih2, b_hh2)
    shared["bi1c"] = _f32(c1.reshape(MC, 128).T)
    shared["bi2c"] = _f32(c2.reshape(MC, 128).T)
    shared["gidec"] = _bf16(
        np.broadcast_to(c1.reshape(MC, 128).T[:, :, None], (128, MC, BS)))
    shared["ident"] = _bf16(np.eye(128))
    shared["bhnv1"] = _bf16(np.asarray(b_hh1, np.float32)[2 * DH:].reshape(1, DH))
    shared["bhnv2"] = _bf16(np.asarray(b_hh2, np.float32)[2 * DH:].reshape(1, DH))
    shared["onesb"] = _bf16(np.ones((1, BS)))
    if cfg["has_out_b"]:
        shared["outb"] = _f8(np.asarray(out_b).reshape(1, -1),
                             PSCALE * HSCALE)
        shared["ones"] = _f8(np.ones((1, 128)), 1.0)

    words = emb[np.asarray(target_variable[:, :TD], dtype=np.int64)]  # [B,TD,DW]

    in_maps = []
    for c in range(NC):
        sl = slice(c * BS, (c + 1) * BS)
        vs = vid_feats[sl]                      # [BS, NF, DV]
        ws = words[sl]                          # [BS, TD, DW]
        m = dict(shared)
        m["xT"] = _bf16(vs.transpose(2, 1, 0).reshape(cfg["DV"], -1))
        m["wordsT"] = _bf16(ws.transpose(2, 1, 0).reshape(cfg["DW"], -1))
        in_maps.append(m)
    return in_maps


_CACHE = {}
LAST_RESULT = None


def kernel(**inputs):
    global LAST_RESULT
    from concourse.bass_utils import run_bass_kernel_spmd

    DH = 512
    has_out_b = bool(np.any(np.asarray(inputs["out_b"])))
    c1 = np.asarray(inputs["b_ih1"], np.float32).copy()
    c1[:2 * DH] += np.asarray(inputs["b_hh1"], np.float32)[:2 * DH]
    has_bias1 = bool(np.any(c1))
    has_bhn1 = bool(np.any(np.asarray(inputs["b_hh1"])[2 * DH:]))
    has_bhn2 = bool(np.any(np.asarray(inputs["b_hh2"])[2 * DH:]))
    key = ("full", has_out_b, has_bias1, has_bhn1, has_bhn2)
    if key not in _CACHE:
        cfg = make_cfg(has_out_b=has_out_b, has_bias1=has_bias1,
                       has_bhn1=has_bhn1, has_bhn2=has_bhn2)
        _CACHE[key] = (cfg, build_nc(cfg))
    cfg, nc = _CACHE[key]

    in_maps = prep_inputs(cfg, **inputs)
    res = run_bass_kernel_spmd(nc, in_maps, list(range(cfg["n_cores"])))
    LAST_RESULT = res
    outs = [np.asarray(res.results[c]["out"]) for c in range(cfg["n_cores"])]
    return np.concatenate(outs, axis=0).astype(np.float32)  # [B, TD, V]


# revision 43
# speedup vs baseline: 1.0280x; 1.0280x over previous
"""Trainium2 Bass kernel for the caption-generation module (2-layer GRU
encoder-decoder + vocab projection + log_softmax).

Strategy: data-parallel over batch across 8 NeuronCores (B=128 -> 16 rows
per core, weights replicated).  Per core, transposed layout (feature dim on
SBUF partitions, (time*batch) on the free dim):

  E1:  gi1[t] = x_t @ w_ih1.T for all 40 encoder steps  (one batched matmul)
  C1/C2: h1/h2 chains, 67 sequential steps each, software-pipelined so the
       two chains' gate math interleaves op-by-op on Vector/Scalar while
       the PE runs the other chain's recurrent matmul.  gi (+b) for the
       r/z gates is preloaded into PSUM so the matmuls accumulate onto it
       and sigmoid reads PSUM directly.
  E3:  gi2[t] = [h1_t; w_t] @ w_ih2.T in blocks (batched matmul)
  P :  logits = h2_dec @ out_w.T (fp8), streamed log_softmax in two
       row-tile groups so the first group's output DMA overlaps the
       second group's projection; output written as bf16.
"""

import sys

sys.path.insert(0, "/opt/trn_rl_repo")

import numpy as np
import ml_dtypes

import concourse.bass as bass
import concourse.mybir as mybir
import concourse.tile as tile
from concourse.alu_op_type import AluOpType
from concourse.vector_clock import ScopedClock

# Align the Tile scheduler's PE cost model with measured hardware: a 16-col
# chain matmul sustains ~28ns/instruction on this device (issue-floor bound)
# vs ~7ns modeled at the 2.4GHz peak clock.  The skewed model makes the
# static scheduler bunch matmuls ahead of the gate ops and misorder the
# Scalar stream.  These constants are read lazily (once per process) by the
# rust cost model, so patch before the first build.  Schedule-order only —
# no effect on emitted semantics.
import concourse.hw_specs as _hw_specs

_hw_specs.TRN2Spec.PE_CYCLE = 1e9 / 0.6e9
_hw_specs.TRN2Spec.PE_CYCLE_PSTATE_MID = 1e9 / 0.55e9
_hw_specs.TRN2Spec.PE_CYCLE_PSTATE_LOW = 1e9 / 0.5e9

BF16 = mybir.dt.bfloat16
F32 = mybir.dt.float32
F8 = mybir.dt.float8e3
PSCALE = 64.0   # fp8 out_w pre-scale
HSCALE = 8.0    # fp8 h2 pre-scale
AF = mybir.ActivationFunctionType


# ---------------------------------------------------------------------------
# Workaround: this container's walrus rejects CTRL instructions carrying more
# than one sync-wait command.  Split the TileContext tail drain's wait list
# across a chain of drains, one wait each.
# ---------------------------------------------------------------------------
def _patched_drain_and_barrier(self, tick_clock, wait_clock):
    import bass_rust

    drain_inst = self.nc.sync.drain()
    wait_clock.add_sem_waits(
        drain_inst.ins, ScopedClock({None: tick_clock.global_clock})
    )
    waits = list(drain_inst.ins.sync_info.on_wait)
    if len(waits) > 1:
        si = drain_inst.ins.sync_info
        si.on_wait = waits[:1]
        drain_inst.ins.sync_info = si
        for i in range(1, len(waits)):
            extra = self.nc.sync.drain()
            extra.ins.sync_info = bass_rust.SyncInfo(
                on_wait=waits[i : i + 1], on_update=[]
            )
    self.nc.all_engine_barrier()
    assert self.sems is not None
    popped = self.nc._tile_sem_poison_stack.pop()
    assert popped is self._sem_poison
    self.nc.clear_and_free_semaphores(list(self.sems.allocated().values()))
    self.nc.all_engine_barrier()


tile.TileContext._drain_and_barrier = _patched_drain_and_barrier

# Same walrus limitation for regular engine instructions: at most one
# sync-wait per instruction.  Split extra waits onto preceding NoOps on the
# same engine (engine stalls there instead — identical semantics).
_orig_commit = tile.TileContext._commit_instruction


def _commit_split_waits(self, inst, lazy_reg_writes=True):
    si = getattr(inst, "sync_info", None)
    if (si is not None and si.on_wait and len(si.on_wait) > 1
            and inst.engine != mybir.EngineType.Unassigned):
        waits = list(si.on_wait)
        for w in waits[:-1]:
            nop = mybir.InstNoOp(
                name=self.nc.get_next_instruction_name(),
                sync_info=mybir.SyncInfo(on_wait=[w], on_update=[]),
                bass_nofuse=True,
                engine=inst.engine,
            )
            _orig_commit(self, nop, lazy_reg_writes=False)
        si.on_wait = waits[-1:]
        inst.sync_info = si
    return _orig_commit(self, inst, lazy_reg_writes)


tile.TileContext._commit_instruction = _commit_split_waits


# ---------------------------------------------------------------------------
# Config
# ---------------------------------------------------------------------------
def make_cfg(B=128, NF=40, TD=27, V=16000, DV=2048, DH=512, DW=512,
             n_cores=8, has_out_b=False, has_bias1=False, has_bhn1=False,
             has_bhn2=False):
    cfg = dict(B=B, NF=NF, TD=TD, V=V, DV=DV, DH=DH, DW=DW,
               n_cores=n_cores, has_out_b=has_out_b, has_bias1=has_bias1,
               has_bhn1=has_bhn1, has_bhn2=has_bhn2)
    cfg["BS"] = B // n_cores          # batch rows per core
    cfg["KV"] = DV // 128             # x feature chunks
    cfg["KH"] = DH // 128             # h feature chunks
    cfg["KW"] = DW // 128             # word feature chunks
    cfg["MC"] = 3 * DH // 128         # gate chunks
    cfg["NSTEP"] = NF + TD            # total chain steps
    cfg["ROWS_E"] = NF * cfg["BS"]    # encoder (t,b) columns
    cfg["ROWS_A"] = cfg["NSTEP"] * cfg["BS"]
    cfg["ROWS_D"] = TD * cfg["BS"]    # decode (t,b) columns
    # vocab tiling for the projection (psum free dim <= 512 fp32)
    for pn in (512, 500, 400, 320, 256):
        if V % pn == 0:
            cfg["PN"] = pn
            break
    else:
        raise ValueError(f"V={V} has no tile size")
    return cfg


def _ntiles(total, maxn):
    """Split `total` into tiles of at most maxn (last ragged)."""
    out = []
    n0 = 0
    while n0 < total:
        nn = min(maxn, total - n0)
        out.append((n0, nn))
        n0 += nn
    return out


# ---------------------------------------------------------------------------
# Kernel builder
# ---------------------------------------------------------------------------
def build_nc(cfg):
    BS, KV, KH, KW, MC = cfg["BS"], cfg["KV"], cfg["KH"], cfg["KW"], cfg["MC"]
    NF, TD, V, DH = cfg["NF"], cfg["TD"], cfg["V"], cfg["DH"]
    NSTEP, ROWS_E, ROWS_D = cfg["NSTEP"], cfg["ROWS_E"], cfg["ROWS_D"]
    PN = cfg["PN"]
    G3 = 3 * DH
    RZ = 2 * KH   # number of r+z gate chunks (psum-preloaded with gi)
    LAG = 12      # h2 chain trails h1 by this many steps (> E3 block size)

    nc = bass.Bass()

    # ---- DRAM parameters (per-core views; host prepares these) ----
    xT = nc.dram_tensor("xT", [cfg["DV"], ROWS_E], BF16, kind="ExternalInput")
    wordsT = nc.dram_tensor("wordsT", [cfg["DW"], ROWS_D], BF16, kind="ExternalInput")
    w1T = nc.dram_tensor("w1T", [cfg["DV"], G3], BF16, kind="ExternalInput")
    wh1T = nc.dram_tensor("wh1T", [DH, G3], BF16, kind="ExternalInput")
    w2T = nc.dram_tensor("w2T", [DH + cfg["DW"], G3], BF16, kind="ExternalInput")
    wh2T = nc.dram_tensor("wh2T", [DH, G3], BF16, kind="ExternalInput")
    owT = nc.dram_tensor("owT", [DH, V], F8, kind="ExternalInput")
    bi1c = nc.dram_tensor("bi1c", [128, MC], F32, kind="ExternalInput")
    bi2c = nc.dram_tensor("bi2c", [128, MC], F32, kind="ExternalInput")
    gidec = nc.dram_tensor("gidec", [128, MC, BS], BF16, kind="ExternalInput")
    ident = nc.dram_tensor("ident", [128, 128], BF16, kind="ExternalInput")
    bhnv1 = nc.dram_tensor("bhnv1", [1, DH], BF16, kind="ExternalInput")
    bhnv2 = nc.dram_tensor("bhnv2", [1, DH], BF16, kind="ExternalInput")
    onesb = nc.dram_tensor("onesb", [1, BS], BF16, kind="ExternalInput")
    if cfg["has_out_b"]:
        outb = nc.dram_tensor("outb", [1, V], F8, kind="ExternalInput")
        ones = nc.dram_tensor("ones", [1, 128], F8, kind="ExternalInput")
    out = nc.dram_tensor("out", [BS, TD, V], BF16, kind="ExternalOutput")
    # view [t, b, v] of out[b, t, v] (strides V, TD*V, 1); row r = t*BS + b
    _o = out[:]
    out_tbv = bass.AP(tensor=_o.tensor, offset=_o.offset,
                      ap=[[V, TD], [TD * V, BS], [1, V]])

    def out_slice(r0, mrows, c0, cw):
        assert r0 % BS == 0 and mrows % BS == 0
        return out_tbv[r0 // BS:(r0 + mrows) // BS, :, c0:c0 + cw]

    from contextlib import ExitStack

    with tile.TileContext(nc) as tc:
        with (
            tc.tile_pool(name="pconst", bufs=1) as pconst,
            tc.tile_pool(name="pchain", bufs=8) as pchain,
            tc.tile_pool(name="psum_mm", bufs=3, space="PSUM") as psum_mm,
            tc.tile_pool(name="psum_gh", bufs=5, space="PSUM") as psum_gh,
            tc.tile_pool(name="ph2", bufs=1) as ph2,
        ):
            # ---- constants ----
            bi1c_sb = pconst.tile([128, MC], F32, tag="bi1c")
            nc.sync.dma_start(out=bi1c_sb[:], in_=bi1c[:])
            bi2c_sb = pconst.tile([128, MC], F32, tag="bi2c")
            nc.sync.dma_start(out=bi2c_sb[:], in_=bi2c[:])
            gidec_sb = pconst.tile([128, MC, BS], BF16, tag="gidec")
            nc.sync.dma_start(out=gidec_sb[:], in_=gidec[:])
            ident_sb = pconst.tile([128, 128], BF16, tag="ident")
            nc.sync.dma_start(out=ident_sb[:], in_=ident[:])
            bhnv1_sb = pconst.tile([1, DH], BF16, tag="bhnv1")
            nc.sync.dma_start(out=bhnv1_sb[:], in_=bhnv1[:])
            bhnv2_sb = pconst.tile([1, DH], BF16, tag="bhnv2")
            nc.sync.dma_start(out=bhnv2_sb[:], in_=bhnv2[:])
            onesb_sb = pconst.tile([1, BS], BF16, tag="onesb")
            nc.sync.dma_start(out=onesb_sb[:], in_=onesb[:])
            if cfg["has_out_b"]:
                outb_sb = pconst.tile([1, V], F8, tag="outb")
                nc.sync.dma_start(out=outb_sb[:], in_=outb[:])
                ones_sb = pconst.tile([1, 128], F8, tag="ones")
                nc.sync.dma_start(out=ones_sb[:], in_=ones[:])

            h2_sb = ph2.tile([128, KH, (NSTEP + 1) * BS], BF16, tag="h2")
            nc.vector.memset(h2_sb[:, :, 0:BS], 0.0)

            # ====== E1, interleaved h1/E3/h2 chains ======
            with ExitStack() as chain_es:
                pmidA = chain_es.enter_context(tc.tile_pool(name="pmidA", bufs=1))
                h1_sb = pmidA.tile([128, KH, (NSTEP + 1) * BS], BF16, tag="h1")
                nc.vector.memset(h1_sb[:, :, 0:BS], 0.0)
                gi1_sb = pmidA.tile([128, MC, ROWS_E], BF16, tag="gi1")
                wh1_sb = pmidA.tile([128, KH, G3], BF16, tag="wh1")

                with tc.tile_pool(name="pw1", bufs=1) as pw1:
                    x_sb = pw1.tile([128, KV, ROWS_E], BF16, tag="x")
                    xT_r = xT[:].rearrange("(k p) n -> p k n", p=128)
                    for k in range(KV):
                        nc.sync.dma_start(out=x_sb[:, k, :], in_=xT_r[:, k, :])
                    w1_sb = pw1.tile([128, KV, G3], BF16, tag="w1")
                    w1T_r = w1T[:].rearrange("(k p) n -> p k n", p=128)
                    for k in range(KV):
                        nc.sync.dma_start(out=w1_sb[:, k, :], in_=w1T_r[:, k, :])
                    wh1T_r = wh1T[:].rearrange("(k p) n -> p k n", p=128)
                    for k in range(KH):
                        nc.sync.dma_start(out=wh1_sb[:, k, :], in_=wh1T_r[:, k, :])

                    # E1: gi1 = w1T.T @ x  (+ bias via ACT copy)
                    for (n0, nn) in _ntiles(ROWS_E, 320):
                        for m in range(MC):
                            ps = psum_mm.tile([128, 512], F32, tag="mm")
                            for k in range(KV):
                                nc.tensor.matmul(
                                    ps[:, :nn],
                                    lhsT=w1_sb[:, k, m * 128:(m + 1) * 128],
                                    rhs=x_sb[:, k, n0:n0 + nn],
                                    start=(k == 0), stop=(k == KV - 1))
                            nc.scalar.activation(
                                out=gi1_sb[:, m, n0:n0 + nn], in_=ps[:, :nn],
                                func=AF.Identity, bias=bi1c_sb[:, m:m + 1],
                                scale=1.0)

                # layer-2 weights / words / gi2 (loaded while chains run)
                pmidB = chain_es.enter_context(tc.tile_pool(name="pmidB", bufs=1))
                w2_sb = pmidB.tile([128, KH + KW, G3], BF16, tag="w2")
                w2T_r = w2T[:].rearrange("(k p) n -> p k n", p=128)
                for k in range(KH + KW):
                    nc.sync.dma_start(out=w2_sb[:, k, :], in_=w2T_r[:, k, :])
                words_sb = pmidB.tile([128, KW, ROWS_D], BF16, tag="words")
                wordsT_r = wordsT[:].rearrange("(k p) n -> p k n", p=128)
                for k in range(KW):
                    nc.sync.dma_start(out=words_sb[:, k, :], in_=wordsT_r[:, k, :])
                wh2_sb = pmidB.tile([128, KH, G3], BF16, tag="wh2")
                wh2T_r = wh2T[:].rearrange("(k p) n -> p k n", p=128)
                for k in range(KH):
                    nc.sync.dma_start(out=wh2_sb[:, k, :], in_=wh2T_r[:, k, :])
                gi2_sb = pmidB.tile([128, MC, NSTEP * BS], BF16, tag="gi2")

                # ---------------- chain machinery -------------------------
                # psum gh tile: one identity-matmul injects gi_rz (start=True
                # for the whole r/z region, so each m-region's accumulation
                # group closes at its own stop and consumers get fine-grained
                # waits: sigmoid fires right after the r-chunk matmuls).
                # m order: r chunks first, then n (np0), then z (hm1, last).
                M_ORDER = (list(range(KH)) + list(range(RZ, MC))
                           + list(range(KH, RZ)))
                gh_live = {}

                def girz_of(cid, t):
                    if cid == 1:
                        if t < NF:
                            return gi1_sb[:, 0:RZ, t * BS:(t + 1) * BS]
                        return gidec_sb[:, 0:RZ, :] if cfg["has_bias1"] else None
                    return gi2_sb[:, 0:RZ, t * BS:(t + 1) * BS]

                def gin_of(cid, t):
                    if cid == 1:
                        if t < NF:
                            return gi1_sb[:, RZ:, t * BS:(t + 1) * BS]
                        return gidec_sb[:, RZ:, :] if cfg["has_bias1"] else None
                    return gi2_sb[:, RZ:, t * BS:(t + 1) * BS]

                def chain_mms(cid, t):
                    gh = psum_gh.tile([128, MC, BS], F32, tag="gh")
                    gh_live[(cid, t)] = gh
                    whh = wh1_sb if cid == 1 else wh2_sb
                    hsb = h1_sb if cid == 1 else h2_sb
                    has_bhn = cfg["has_bhn1"] if cid == 1 else cfg["has_bhn2"]
                    bhnv_sb = bhnv1_sb if cid == 1 else bhnv2_sb
                    prev = hsb[:, :, t * BS:(t + 1) * BS]
                    inj = girz_of(cid, t)
                    if inj is not None:
                        nc.tensor.matmul(
                            gh[:, 0:RZ, :], lhsT=ident_sb[:], rhs=inj,
                            start=True, stop=False, skip_group_check=True)
                    for m in M_ORDER:
                        is_rz = m < RZ
                        if not is_rz and has_bhn:
                            # rank-1: gh_n += bhn_chunk (x) ones
                            nc.tensor.matmul(
                                gh[:, m, :],
                                lhsT=bhnv_sb[0:1, (m - RZ) * 128:(m - RZ + 1) * 128],
                                rhs=onesb_sb[0:1, :],
                                start=True, stop=False, skip_group_check=True)
                        for k in range(KH):
                            start = (k == 0) and (
                                (is_rz and inj is None)
                                or (not is_rz and not has_bhn))
                            nc.tensor.matmul(
                                gh[:, m, :],
                                lhsT=whh[:, k, m * 128:(m + 1) * 128],
                                rhs=prev[:, k, :],
                                start=start, stop=(k == KH - 1),
                                skip_group_check=True)

                def gate_sig_r(cid, t, st):
                    gh = gh_live[(cid, t)]
                    rzs = pchain.tile([128, RZ, BS], F32, tag=f"rzs{cid}")
                    nc.scalar.activation(out=rzs[:, 0:KH, :],
                                         in_=gh[:, 0:KH, :], func=AF.Sigmoid)
                    st["rzs"] = rzs

                def gate_sig_z(cid, t, st):
                    # emitted after tanh: z is needed only by hm1, so it must
                    # not sit between np0 and tanh in the Scalar stream
                    gh = gh_live[(cid, t)]
                    nc.scalar.activation(out=st["rzs"][:, KH:RZ, :],
                                         in_=gh[:, KH:RZ, :], func=AF.Sigmoid)

                def gate_np(cid, t, st):
                    # np0 reads the n-gate psum directly (includes bhn via the
                    # rank-1 matmul when nonzero); np1 adds gi_n when present.
                    # np0 reads PSUM so it stays on Vector for both chains;
                    # chain 2's remaining SBUF-only ops go to idle GpSimd.
                    gh = gh_live[(cid, t)]
                    eng1 = nc.vector
                    eng2 = nc.vector if cid == 1 else nc.gpsimd
                    np0 = pchain.tile([128, KH, BS], F32, tag=f"np0{cid}")
                    eng1.tensor_tensor(
                        out=np0[:], in0=st["rzs"][:, 0:KH, :],
                        in1=gh[:, RZ:, :], op=AluOpType.mult)
                    gin = gin_of(cid, t)
                    if gin is None:
                        st["np1"] = np0
                    else:
                        np1 = pchain.tile([128, KH, BS], F32, tag=f"np1{cid}")
                        eng2.tensor_tensor(
                            out=np1[:], in0=gin, in1=np0[:],
                            op=AluOpType.add)
                        st["np1"] = np1

                def gate_tanh(cid, st):
                    nt = pchain.tile([128, KH, BS], F32, tag=f"nt{cid}")
                    nc.scalar.activation(out=nt[:], in_=st["np1"][:],
                                         func=AF.Tanh)
                    st["nt"] = nt

                def gate_out(cid, t, st):
                    eng = nc.vector if cid == 1 else nc.gpsimd
                    hsb = h1_sb if cid == 1 else h2_sb
                    prev = hsb[:, :, t * BS:(t + 1) * BS]
                    hm0 = pchain.tile([128, KH, BS], F32, tag=f"hm0{cid}")
                    eng.tensor_tensor(
                        out=hm0[:], in0=prev[:], in1=st["nt"][:],
                        op=AluOpType.subtract)
                    hm1 = pchain.tile([128, KH, BS], F32, tag=f"hm1{cid}")
                    eng.tensor_tensor(
                        out=hm1[:], in0=st["rzs"][:, KH:RZ, :], in1=hm0[:],
                        op=AluOpType.mult)
                    eng.tensor_tensor(
                        out=hsb[:, :, (t + 1) * BS:(t + 2) * BS],
                        in0=st["nt"][:], in1=hm1[:], op=AluOpType.add)

                def e3_block(t0, nsteps):
                    """gi2 for chain steps [t0, t0+nsteps); drains split S/V."""
                    n0 = t0 * BS
                    nn = nsteps * BS
                    enc = t0 < NF  # blocks never straddle NF
                    for m in range(MC):
                        ps = psum_mm.tile([128, 512], F32, tag="mm")
                        for k in range(KH):
                            nc.tensor.matmul(
                                ps[:, :nn],
                                lhsT=w2_sb[:, k, m * 128:(m + 1) * 128],
                                rhs=h1_sb[:, k, BS + n0:BS + n0 + nn],
                                start=(k == 0),
                                stop=(enc and k == KH - 1))
                        if not enc:
                            w0 = n0 - ROWS_E
                            for k in range(KW):
                                nc.tensor.matmul(
                                    ps[:, :nn],
                                    lhsT=w2_sb[:, KH + k, m * 128:(m + 1) * 128],
                                    rhs=words_sb[:, k, w0:w0 + nn],
                                    start=False, stop=(k == KW - 1))
                        # drains on Vector only: Scalar carries the chains'
                        # sigmoid/tanh critical path
                        nc.vector.tensor_scalar_add(
                            gi2_sb[:, m, n0:n0 + nn], ps[:, :nn],
                            bi2c_sb[:, m:m + 1])

                blocks = ([(t0, nn) for (t0, nn) in _ntiles(NF, 8)] +
                          [(NF + t0, nn) for (t0, nn) in _ntiles(TD, 9)])
                block_end = {t0 + nn: (t0, nn) for (t0, nn) in blocks}

                # Emission order per pair is chosen so each engine's in-order
                # stream never blocks on the other chain's not-yet-ready op:
                #   PE: h1 mms, h2 mms, (e3)
                #   S : sigr1 sigz1 tanh1 sigr2 sigz2 tanh2
                #   V : preloads(next) tmpn1 np0_1 np1_1 tmpn2 hm_1* np_2* hm_2*
                st1, st2 = {}, {}
                for tt in range(NSTEP + LAG):
                    t1 = tt if tt < NSTEP else None
                    s2 = tt - LAG if 0 <= tt - LAG < NSTEP else None
                    if t1 is not None:
                        chain_mms(1, t1)
                    if s2 is not None:
                        chain_mms(2, s2)
                    if t1 is not None:
                        gate_sig_r(1, t1, st1)
                        gate_np(1, t1, st1)
                        gate_tanh(1, st1)
                        gate_sig_z(1, t1, st1)
                        gate_out(1, t1, st1)
                        del gh_live[(1, t1)]
                    if s2 is not None:
                        gate_sig_r(2, s2, st2)
                        gate_np(2, s2, st2)
                        gate_tanh(2, st2)
                        gate_sig_z(2, s2, st2)
                        gate_out(2, s2, st2)
                        del gh_live[(2, s2)]
                    if t1 is not None and t1 + 1 in block_end:
                        e3_block(*block_end[t1 + 1])

            # ---- projection + log_softmax over decode steps ----
            # |logits| is bounded well below fp32 exp overflow here, so
            # log_softmax runs without the max shift: lp = x - ln(sum(e^x)).
            # Two row-tile groups: group 0's lse/subtract/output-DMA overlap
            # group 1's projection.  out_w and h2 in fp8 (pre-scaled).
            with (
                tc.tile_pool(name="pp", bufs=1) as pp,
                tc.tile_pool(name="pwst", bufs=3) as pwst,
                tc.tile_pool(name="pstage", bufs=3) as pstage,
                tc.tile_pool(name="psmall", bufs=2) as psmall,
            ):
                owT_r = owT[:].rearrange("(k p) n -> p k n", p=128)
                nvt = V // PN
                dcol0 = (NF + 1) * BS  # first decode h2 col
                mtiles = _ntiles(ROWS_D, 128)
                inv_s = 1.0 / (PSCALE * HSCALE)
                # h2 decode block, fp8 pre-scaled by HSCALE
                h2f8 = pp.tile([128, KH, ROWS_D], F8, tag="h2f8")
                for k in range(KH):
                    nc.vector.tensor_scalar_mul(
                        h2f8[:, k, :], h2_sb[:, k, dcol0:dcol0 + ROWS_D],
                        HSCALE)
                logits_t = [pp.tile([128, V], BF16, tag=f"logits{i}",
                                    name=f"logits{i}")
                            for i in range(len(mtiles))]
                sums_t = [psmall.tile([128, nvt], F32, tag=f"sums{i}",
                                      name=f"sums{i}")
                          for i in range(len(mtiles))]
                NCH = 4
                CW = V // NCH
                for g0 in range(0, len(mtiles), 2):
                    grp = [(mt, mtiles[mt])
                           for mt in range(g0, min(g0 + 2, len(mtiles)))]
                    for nt_i in range(nvt):
                        n0 = nt_i * PN
                        wst = pwst.tile([128, KH, PN], F8, tag="wst")
                        nc.sync.dma_start(out=wst[:], in_=owT_r[:, :, n0:n0 + PN])
                        for mt, (r0, mrows) in grp:
                            ps = psum_mm.tile([128, 512], F32, tag="mm")
                            last = KH - 1 if not cfg["has_out_b"] else None
                            for k in range(KH):
                                nc.tensor.matmul(
                                    ps[:mrows, :PN],
                                    lhsT=h2f8[:, k, r0:r0 + mrows],
                                    rhs=wst[:, k, :],
                                    start=(k == 0), stop=(k == last))
                            if cfg["has_out_b"]:
                                nc.tensor.matmul(
                                    ps[:mrows, :PN],
                                    lhsT=ones_sb[:, :mrows],
                                    rhs=outb_sb[:, n0:n0 + PN],
                                    start=False, stop=True)
                            edump = pstage.tile([128, PN], BF16, tag="edump")
                            nc.scalar.activation(
                                out=edump[:mrows, :], in_=ps[:mrows, :PN],
                                func=AF.Exp, scale=inv_s,
                                accum_out=sums_t[mt][:mrows, nt_i:nt_i + 1])
                            nc.vector.tensor_scalar_mul(
                                logits_t[mt][:mrows, n0:n0 + PN],
                                ps[:mrows, :PN], inv_s)
                    # group tail: lse per row tile, then logp = logits - lse,
                    # alternating ACT / DVE per chunk; bf16 output via
                    # alternating gpsimd / sync DMA queues
                    for mt, (r0, mrows) in grp:
                        s1 = psmall.tile([128, 1], F32, tag="s1")
                        nc.vector.tensor_reduce(
                            out=s1[:mrows], in_=sums_t[mt][:mrows, :],
                            axis=mybir.AxisListType.X, op=AluOpType.add)
                        nshift = psmall.tile([128, 1], F32, tag="nshift")
                        nc.scalar.activation(
                            out=nshift[:mrows], in_=s1[:mrows], func=AF.Ln)
                        nc.vector.tensor_scalar_mul(
                            nshift[:mrows], nshift[:mrows], -1.0)
                        for c in range(NCH):
                            stage = pstage.tile([128, CW], BF16, tag="stage")
                            src = logits_t[mt][:mrows, c * CW:(c + 1) * CW]
                            if c % 2 == 0:
                                nc.scalar.activation(
                                    out=stage[:mrows, :], in_=src,
                                    func=AF.Identity, bias=nshift[:mrows])
                            else:
                                nc.vector.tensor_scalar_add(
                                    stage[:mrows, :], src, nshift[:mrows])
                            eng = nc.gpsimd if c % 2 == 0 else nc.sync
                            eng.dma_start(
                                out=out_slice(r0, mrows, c * CW, CW),
                                in_=stage[:mrows, :])
    return nc


# ---------------------------------------------------------------------------
# Host side
# ---------------------------------------------------------------------------
def _bf16(a):
    return np.ascontiguousarray(a, dtype=np.float32).astype(ml_dtypes.bfloat16)


def _f32(a):
    return np.ascontiguousarray(a, dtype=np.float32)


def _f8(a, scale):
    f8 = ml_dtypes.float8_e3m4
    f8max = float(ml_dtypes.finfo(f8).max)
    scaled = np.clip(np.asarray(a, dtype=np.float32) * scale, -f8max, f8max)
    return np.ascontiguousarray(scaled).astype(f8)


def prep_inputs(cfg, vid_feats, target_variable, emb, w_ih1, w_hh1, b_ih1,
                b_hh1, w_ih2, w_hh2, b_ih2, b_hh2, out_w, out_b):
    """Build per-core input maps."""
    BS, MC, KH, DH = cfg["BS"], cfg["MC"], cfg["KH"], cfg["DH"]
    TD, NC = cfg["TD"], cfg["n_cores"]

    vid_feats = np.asarray(vid_feats, dtype=np.float32)
    target_variable = np.asarray(target_variable)
    emb = np.asarray(emb, dtype=np.float32)

    shared = {
        "w1T": _bf16(np.asarray(w_ih1).T),
        "wh1T": _bf16(np.asarray(w_hh1).T),
        "w2T": _bf16(np.asarray(w_ih2).T),
        "wh2T": _bf16(np.asarray(w_hh2).T),
        "owT": _f8(np.asarray(out_w).T, PSCALE),
    }
    # combined biases: b_ih (+ b_hh for the r,z chunks; the n chunk of b_hh
    # is applied inside the gate, before the r multiply)
    def comb(bi, bh):
        c = np.asarray(bi, dtype=np.float32).copy()
        c[: 2 * DH] += np.asarray(bh, dtype=np.float32)[: 2 * DH]
        return c

    c1 = comb(b_ih1, b_hh1)
    c2 = comb(b_ih2, b_hh2)
    shared["bi1c"] = _f32(c1.reshape(MC, 128).T)
    shared["bi2c"] = _f32(c2.reshape(MC, 128).T)
    shared["gidec"] = _bf16(
        np.broadcast_to(c1.reshape(MC, 128).T[:, :, None], (128, MC, BS)))
    shared["ident"] = _bf16(np.eye(128))
    shared["bhnv1"] = _bf16(np.asarray(b_hh1, np.float32)[2 * DH:].reshape(1, DH))
    shared["bhnv2"] = _bf16(np.asarray(b_hh2, np.float32)[2 * DH:].reshape(1, DH))
    shared["onesb"] = _bf16(np.ones((1, BS)))
    if cfg["has_out_b"]:
        shared["outb"] = _f8(np.asarray(out_b).reshape(1, -1),
                             PSCALE * HSCALE)
        shared["ones"] = _f8(np.ones((1, 128)), 1.0)

    words = emb[np.asarray(target_variable[:, :TD], dtype=np.int64)]  # [B,TD,DW]

    in_maps = []
    for c in range(NC):
        sl = slice(c * BS, (c + 1) * BS)
        vs = vid_feats[sl]                      # [BS, NF, DV]
        ws = words[sl]                          # [BS, TD, DW]
        m = dict(shared)
        m["xT"] = _bf16(vs.transpose(2, 1, 0).reshape(cfg["DV"], -1))
        m["wordsT"] = _bf16(ws.transpose(2, 1, 0).reshape(cfg["DW"], -1))
        in_maps.append(m)
    return in_maps


_CACHE = {}
LAST_RESULT = None


def kernel(**inputs):
    global LAST_RESULT
    from concourse.bass_utils import run_bass_kernel_spmd

    DH = 512
    has_out_b = bool(np.any(np.asarray(inputs["out_b"])))
    c1 = np.asarray(inputs["b_ih1"], np.float32).copy()
    c1[:2 * DH] += np.asarray(inputs["b_hh1"], np.float32)[:2 * DH]
    has_bias1 = bool(np.any(c1))
    has_bhn1 = bool(np.any(np.asarray(inputs["b_hh1"])[2 * DH:]))
    has_bhn2 = bool(np.any(np.asarray(inputs["b_hh2"])[2 * DH:]))
    key = ("full", has_out_b, has_bias1, has_bhn1, has_bhn2)
    if key not in _CACHE:
        cfg = make_cfg(has_out_b=has_out_b, has_bias1=has_bias1,
                       has_bhn1=has_bhn1, has_bhn2=has_bhn2)
        _CACHE[key] = (cfg, build_nc(cfg))
    cfg, nc = _CACHE[key]

    in_maps = prep_inputs(cfg, **inputs)
    res = run_bass_kernel_spmd(nc, in_maps, list(range(cfg["n_cores"])))
    LAST_RESULT = res
    outs = [np.asarray(res.results[c]["out"]) for c in range(cfg["n_cores"])]
    return np.concatenate(outs, axis=0).astype(np.float32)  # [B, TD, V]


# revision 44
# speedup vs baseline: 1.0492x; 1.0205x over previous
"""Trainium2 Bass kernel for the caption-generation module (2-layer GRU
encoder-decoder + vocab projection + log_softmax).

Strategy: data-parallel over batch across 8 NeuronCores (B=128 -> 16 rows
per core, weights replicated).  Per core, transposed layout (feature dim on
SBUF partitions, (time*batch) on the free dim):

  E1:  gi1[t] = x_t @ w_ih1.T for all 40 encoder steps  (one batched matmul)
  C1/C2: h1/h2 chains, 67 sequential steps each, software-pipelined so the
       two chains' gate math interleaves op-by-op on Vector/Scalar while
       the PE runs the other chain's recurrent matmul.  gi (+b) for the
       r/z gates is preloaded into PSUM so the matmuls accumulate onto it
       and sigmoid reads PSUM directly.
  E3:  gi2[t] = [h1_t; w_t] @ w_ih2.T in blocks (batched matmul)
  P :  logits = h2_dec @ out_w.T (fp8), streamed log_softmax in two
       row-tile groups so the first group's output DMA overlaps the
       second group's projection; output written as bf16.
"""

import sys

sys.path.insert(0, "/opt/trn_rl_repo")

import numpy as np
import ml_dtypes

import concourse.bass as bass
import concourse.mybir as mybir
import concourse.tile as tile
from concourse.alu_op_type import AluOpType
from concourse.vector_clock import ScopedClock

# Align the Tile scheduler's PE cost model with measured hardware: a 16-col
# chain matmul sustains ~28ns/instruction on this device (issue-floor bound)
# vs ~7ns modeled at the 2.4GHz peak clock.  The skewed model makes the
# static scheduler bunch matmuls ahead of the gate ops and misorder the
# Scalar stream.  These constants are read lazily (once per process) by the
# rust cost model, so patch before the first build.  Schedule-order only —
# no effect on emitted semantics.
import concourse.hw_specs as _hw_specs

_hw_specs.TRN2Spec.PE_CYCLE = 1e9 / 0.6e9
_hw_specs.TRN2Spec.PE_CYCLE_PSTATE_MID = 1e9 / 0.55e9
_hw_specs.TRN2Spec.PE_CYCLE_PSTATE_LOW = 1e9 / 0.5e9

BF16 = mybir.dt.bfloat16
F32 = mybir.dt.float32
F8 = mybir.dt.float8e3
F8E4 = mybir.dt.float8e4
PSCALE = 64.0   # fp8 out_w pre-scale
HSCALE = 8.0    # fp8 h2 pre-scale
AF = mybir.ActivationFunctionType


# ---------------------------------------------------------------------------
# Workaround: this container's walrus rejects CTRL instructions carrying more
# than one sync-wait command.  Split the TileContext tail drain's wait list
# across a chain of drains, one wait each.
# ---------------------------------------------------------------------------
def _patched_drain_and_barrier(self, tick_clock, wait_clock):
    import bass_rust

    drain_inst = self.nc.sync.drain()
    wait_clock.add_sem_waits(
        drain_inst.ins, ScopedClock({None: tick_clock.global_clock})
    )
    waits = list(drain_inst.ins.sync_info.on_wait)
    if len(waits) > 1:
        si = drain_inst.ins.sync_info
        si.on_wait = waits[:1]
        drain_inst.ins.sync_info = si
        for i in range(1, len(waits)):
            extra = self.nc.sync.drain()
            extra.ins.sync_info = bass_rust.SyncInfo(
                on_wait=waits[i : i + 1], on_update=[]
            )
    self.nc.all_engine_barrier()
    assert self.sems is not None
    popped = self.nc._tile_sem_poison_stack.pop()
    assert popped is self._sem_poison
    self.nc.clear_and_free_semaphores(list(self.sems.allocated().values()))
    self.nc.all_engine_barrier()


tile.TileContext._drain_and_barrier = _patched_drain_and_barrier

# Same walrus limitation for regular engine instructions: at most one
# sync-wait per instruction.  Split extra waits onto preceding NoOps on the
# same engine (engine stalls there instead — identical semantics).
_orig_commit = tile.TileContext._commit_instruction


def _commit_split_waits(self, inst, lazy_reg_writes=True):
    si = getattr(inst, "sync_info", None)
    if (si is not None and si.on_wait and len(si.on_wait) > 1
            and inst.engine != mybir.EngineType.Unassigned):
        waits = list(si.on_wait)
        for w in waits[:-1]:
            nop = mybir.InstNoOp(
                name=self.nc.get_next_instruction_name(),
                sync_info=mybir.SyncInfo(on_wait=[w], on_update=[]),
                bass_nofuse=True,
                engine=inst.engine,
            )
            _orig_commit(self, nop, lazy_reg_writes=False)
        si.on_wait = waits[-1:]
        inst.sync_info = si
    return _orig_commit(self, inst, lazy_reg_writes)


tile.TileContext._commit_instruction = _commit_split_waits


# ---------------------------------------------------------------------------
# Config
# ---------------------------------------------------------------------------
def make_cfg(B=128, NF=40, TD=27, V=16000, DV=2048, DH=512, DW=512,
             n_cores=8, has_out_b=False, has_bias1=False, has_bhn1=False,
             has_bhn2=False):
    cfg = dict(B=B, NF=NF, TD=TD, V=V, DV=DV, DH=DH, DW=DW,
               n_cores=n_cores, has_out_b=has_out_b, has_bias1=has_bias1,
               has_bhn1=has_bhn1, has_bhn2=has_bhn2)
    cfg["BS"] = B // n_cores          # batch rows per core
    cfg["KV"] = DV // 128             # x feature chunks
    cfg["KH"] = DH // 128             # h feature chunks
    cfg["KW"] = DW // 128             # word feature chunks
    cfg["MC"] = 3 * DH // 128         # gate chunks
    cfg["NSTEP"] = NF + TD            # total chain steps
    cfg["ROWS_E"] = NF * cfg["BS"]    # encoder (t,b) columns
    cfg["ROWS_A"] = cfg["NSTEP"] * cfg["BS"]
    cfg["ROWS_D"] = TD * cfg["BS"]    # decode (t,b) columns
    # vocab tiling for the projection (psum free dim <= 512 fp32)
    for pn in (512, 500, 400, 320, 256):
        if V % pn == 0:
            cfg["PN"] = pn
            break
    else:
        raise ValueError(f"V={V} has no tile size")
    return cfg


def _ntiles(total, maxn):
    """Split `total` into tiles of at most maxn (last ragged)."""
    out = []
    n0 = 0
    while n0 < total:
        nn = min(maxn, total - n0)
        out.append((n0, nn))
        n0 += nn
    return out


# ---------------------------------------------------------------------------
# Kernel builder
# ---------------------------------------------------------------------------
def build_nc(cfg):
    BS, KV, KH, KW, MC = cfg["BS"], cfg["KV"], cfg["KH"], cfg["KW"], cfg["MC"]
    NF, TD, V, DH = cfg["NF"], cfg["TD"], cfg["V"], cfg["DH"]
    NSTEP, ROWS_E, ROWS_D = cfg["NSTEP"], cfg["ROWS_E"], cfg["ROWS_D"]
    PN = cfg["PN"]
    G3 = 3 * DH
    RZ = 2 * KH   # number of r+z gate chunks (psum-preloaded with gi)
    LAG = 12      # h2 chain trails h1 by this many steps (> E3 block size)

    nc = bass.Bass()

    # ---- DRAM parameters (per-core views; host prepares these) ----
    xT = nc.dram_tensor("xT", [cfg["DV"], ROWS_E], BF16, kind="ExternalInput")
    wordsT = nc.dram_tensor("wordsT", [cfg["DW"], ROWS_D], BF16, kind="ExternalInput")
    w1T = nc.dram_tensor("w1T", [cfg["DV"], G3], BF16, kind="ExternalInput")
    wh1T = nc.dram_tensor("wh1T", [DH, G3], BF16, kind="ExternalInput")
    w2T = nc.dram_tensor("w2T", [DH + cfg["DW"], G3], BF16, kind="ExternalInput")
    wh2T = nc.dram_tensor("wh2T", [DH, G3], BF16, kind="ExternalInput")
    owT = nc.dram_tensor("owT", [DH, V], F8E4, kind="ExternalInput")
    bi1c = nc.dram_tensor("bi1c", [128, MC], F32, kind="ExternalInput")
    bi2c = nc.dram_tensor("bi2c", [128, MC], F32, kind="ExternalInput")
    gidec = nc.dram_tensor("gidec", [128, MC, BS], BF16, kind="ExternalInput")
    ident = nc.dram_tensor("ident", [128, 128], BF16, kind="ExternalInput")
    bhnv1 = nc.dram_tensor("bhnv1", [1, DH], BF16, kind="ExternalInput")
    bhnv2 = nc.dram_tensor("bhnv2", [1, DH], BF16, kind="ExternalInput")
    onesb = nc.dram_tensor("onesb", [1, BS], BF16, kind="ExternalInput")
    if cfg["has_out_b"]:
        outb = nc.dram_tensor("outb", [1, V], F8, kind="ExternalInput")
        ones = nc.dram_tensor("ones", [1, 128], F8, kind="ExternalInput")
    out = nc.dram_tensor("out", [BS, TD, V], BF16, kind="ExternalOutput")
    # view [t, b, v] of out[b, t, v] (strides V, TD*V, 1); row r = t*BS + b
    _o = out[:]
    out_tbv = bass.AP(tensor=_o.tensor, offset=_o.offset,
                      ap=[[V, TD], [TD * V, BS], [1, V]])

    def out_slice(r0, mrows, c0, cw):
        assert r0 % BS == 0 and mrows % BS == 0
        return out_tbv[r0 // BS:(r0 + mrows) // BS, :, c0:c0 + cw]

    from contextlib import ExitStack

    with tile.TileContext(nc) as tc:
        with (
            tc.tile_pool(name="pconst", bufs=1) as pconst,
            tc.tile_pool(name="pchain", bufs=8) as pchain,
            tc.tile_pool(name="psum_mm", bufs=3, space="PSUM") as psum_mm,
            tc.tile_pool(name="psum_gh", bufs=5, space="PSUM") as psum_gh,
            tc.tile_pool(name="ph2", bufs=1) as ph2,
        ):
            # ---- constants ----
            bi1c_sb = pconst.tile([128, MC], F32, tag="bi1c")
            nc.sync.dma_start(out=bi1c_sb[:], in_=bi1c[:])
            bi2c_sb = pconst.tile([128, MC], F32, tag="bi2c")
            nc.sync.dma_start(out=bi2c_sb[:], in_=bi2c[:])
            gidec_sb = pconst.tile([128, MC, BS], BF16, tag="gidec")
            nc.sync.dma_start(out=gidec_sb[:], in_=gidec[:])
            ident_sb = pconst.tile([128, 128], BF16, tag="ident")
            nc.sync.dma_start(out=ident_sb[:], in_=ident[:])
            bhnv1_sb = pconst.tile([1, DH], BF16, tag="bhnv1")
            nc.sync.dma_start(out=bhnv1_sb[:], in_=bhnv1[:])
            bhnv2_sb = pconst.tile([1, DH], BF16, tag="bhnv2")
            nc.sync.dma_start(out=bhnv2_sb[:], in_=bhnv2[:])
            onesb_sb = pconst.tile([1, BS], BF16, tag="onesb")
            nc.sync.dma_start(out=onesb_sb[:], in_=onesb[:])
            if cfg["has_out_b"]:
                outb_sb = pconst.tile([1, V], F8, tag="outb")
                nc.sync.dma_start(out=outb_sb[:], in_=outb[:])
                ones_sb = pconst.tile([1, 128], F8, tag="ones")
                nc.sync.dma_start(out=ones_sb[:], in_=ones[:])

            h2_sb = ph2.tile([128, KH, (NSTEP + 1) * BS], BF16, tag="h2")
            nc.vector.memset(h2_sb[:, :, 0:BS], 0.0)

            # ====== E1, interleaved h1/E3/h2 chains ======
            with ExitStack() as chain_es:
                pmidA = chain_es.enter_context(tc.tile_pool(name="pmidA", bufs=1))
                h1_sb = pmidA.tile([128, KH, (NSTEP + 1) * BS], BF16, tag="h1")
                nc.vector.memset(h1_sb[:, :, 0:BS], 0.0)
                gi1_sb = pmidA.tile([128, MC, ROWS_E], BF16, tag="gi1")
                wh1_sb = pmidA.tile([128, KH, G3], BF16, tag="wh1")

                with tc.tile_pool(name="pw1", bufs=1) as pw1:
                    x_sb = pw1.tile([128, KV, ROWS_E], BF16, tag="x")
                    xT_r = xT[:].rearrange("(k p) n -> p k n", p=128)
                    for k in range(KV):
                        nc.sync.dma_start(out=x_sb[:, k, :], in_=xT_r[:, k, :])
                    w1_sb = pw1.tile([128, KV, G3], BF16, tag="w1")
                    w1T_r = w1T[:].rearrange("(k p) n -> p k n", p=128)
                    for k in range(KV):
                        nc.sync.dma_start(out=w1_sb[:, k, :], in_=w1T_r[:, k, :])
                    wh1T_r = wh1T[:].rearrange("(k p) n -> p k n", p=128)
                    for k in range(KH):
                        nc.sync.dma_start(out=wh1_sb[:, k, :], in_=wh1T_r[:, k, :])

                    # E1: gi1 = w1T.T @ x  (+ bias via ACT copy)
                    for (n0, nn) in _ntiles(ROWS_E, 320):
                        for m in range(MC):
                            ps = psum_mm.tile([128, 512], F32, tag="mm")
                            for k in range(KV):
                                nc.tensor.matmul(
                                    ps[:, :nn],
                                    lhsT=w1_sb[:, k, m * 128:(m + 1) * 128],
                                    rhs=x_sb[:, k, n0:n0 + nn],
                                    start=(k == 0), stop=(k == KV - 1))
                            nc.scalar.activation(
                                out=gi1_sb[:, m, n0:n0 + nn], in_=ps[:, :nn],
                                func=AF.Identity, bias=bi1c_sb[:, m:m + 1],
                                scale=1.0)

                # layer-2 weights / words / gi2 (loaded while chains run)
                pmidB = chain_es.enter_context(tc.tile_pool(name="pmidB", bufs=1))
                w2_sb = pmidB.tile([128, KH + KW, G3], BF16, tag="w2")
                w2T_r = w2T[:].rearrange("(k p) n -> p k n", p=128)
                for k in range(KH + KW):
                    nc.sync.dma_start(out=w2_sb[:, k, :], in_=w2T_r[:, k, :])
                words_sb = pmidB.tile([128, KW, ROWS_D], BF16, tag="words")
                wordsT_r = wordsT[:].rearrange("(k p) n -> p k n", p=128)
                for k in range(KW):
                    nc.sync.dma_start(out=words_sb[:, k, :], in_=wordsT_r[:, k, :])
                wh2_sb = pmidB.tile([128, KH, G3], BF16, tag="wh2")
                wh2T_r = wh2T[:].rearrange("(k p) n -> p k n", p=128)
                for k in range(KH):
                    nc.sync.dma_start(out=wh2_sb[:, k, :], in_=wh2T_r[:, k, :])
                gi2_sb = pmidB.tile([128, MC, NSTEP * BS], BF16, tag="gi2")

                # ---------------- chain machinery -------------------------
                # psum gh tile: one identity-matmul injects gi_rz (start=True
                # for the whole r/z region, so each m-region's accumulation
                # group closes at its own stop and consumers get fine-grained
                # waits: sigmoid fires right after the r-chunk matmuls).
                # m order: r chunks first, then n (np0), then z (hm1, last).
                M_ORDER = (list(range(KH)) + list(range(RZ, MC))
                           + list(range(KH, RZ)))
                gh_live = {}

                def girz_of(cid, t):
                    if cid == 1:
                        if t < NF:
                            return gi1_sb[:, 0:RZ, t * BS:(t + 1) * BS]
                        return gidec_sb[:, 0:RZ, :] if cfg["has_bias1"] else None
                    return gi2_sb[:, 0:RZ, t * BS:(t + 1) * BS]

                def gin_of(cid, t):
                    if cid == 1:
                        if t < NF:
                            return gi1_sb[:, RZ:, t * BS:(t + 1) * BS]
                        return gidec_sb[:, RZ:, :] if cfg["has_bias1"] else None
                    return gi2_sb[:, RZ:, t * BS:(t + 1) * BS]

                def chain_mms(cid, t):
                    gh = psum_gh.tile([128, MC, BS], F32, tag="gh")
                    gh_live[(cid, t)] = gh
                    whh = wh1_sb if cid == 1 else wh2_sb
                    hsb = h1_sb if cid == 1 else h2_sb
                    has_bhn = cfg["has_bhn1"] if cid == 1 else cfg["has_bhn2"]
                    bhnv_sb = bhnv1_sb if cid == 1 else bhnv2_sb
                    prev = hsb[:, :, t * BS:(t + 1) * BS]
                    inj = girz_of(cid, t)
                    if inj is not None:
                        nc.tensor.matmul(
                            gh[:, 0:RZ, :], lhsT=ident_sb[:], rhs=inj,
                            start=True, stop=False, skip_group_check=True)
                    for m in M_ORDER:
                        is_rz = m < RZ
                        if not is_rz and has_bhn:
                            # rank-1: gh_n += bhn_chunk (x) ones
                            nc.tensor.matmul(
                                gh[:, m, :],
                                lhsT=bhnv_sb[0:1, (m - RZ) * 128:(m - RZ + 1) * 128],
                                rhs=onesb_sb[0:1, :],
                                start=True, stop=False, skip_group_check=True)
                        for k in range(KH):
                            start = (k == 0) and (
                                (is_rz and inj is None)
                                or (not is_rz and not has_bhn))
                            nc.tensor.matmul(
                                gh[:, m, :],
                                lhsT=whh[:, k, m * 128:(m + 1) * 128],
                                rhs=prev[:, k, :],
                                start=start, stop=(k == KH - 1),
                                skip_group_check=True)

                def gate_sig_r(cid, t, st):
                    gh = gh_live[(cid, t)]
                    rzs = pchain.tile([128, RZ, BS], F32, tag=f"rzs{cid}")
                    nc.scalar.activation(out=rzs[:, 0:KH, :],
                                         in_=gh[:, 0:KH, :], func=AF.Sigmoid)
                    st["rzs"] = rzs

                def gate_sig_z(cid, t, st):
                    # emitted after tanh: z is needed only by hm1, so it must
                    # not sit between np0 and tanh in the Scalar stream
                    gh = gh_live[(cid, t)]
                    nc.scalar.activation(out=st["rzs"][:, KH:RZ, :],
                                         in_=gh[:, KH:RZ, :], func=AF.Sigmoid)

                def gate_np(cid, t, st):
                    # np0 reads the n-gate psum directly (includes bhn via the
                    # rank-1 matmul when nonzero); np1 adds gi_n when present.
                    # np0 reads PSUM so it stays on Vector for both chains;
                    # chain 2's remaining SBUF-only ops go to idle GpSimd.
                    gh = gh_live[(cid, t)]
                    eng1 = nc.vector
                    eng2 = nc.vector if cid == 1 else nc.gpsimd
                    np0 = pchain.tile([128, KH, BS], F32, tag=f"np0{cid}")
                    eng1.tensor_tensor(
                        out=np0[:], in0=st["rzs"][:, 0:KH, :],
                        in1=gh[:, RZ:, :], op=AluOpType.mult)
                    gin = gin_of(cid, t)
                    if gin is None:
                        st["np1"] = np0
                    else:
                        np1 = pchain.tile([128, KH, BS], F32, tag=f"np1{cid}")
                        eng2.tensor_tensor(
                            out=np1[:], in0=gin, in1=np0[:],
                            op=AluOpType.add)
                        st["np1"] = np1

                def gate_tanh(cid, st):
                    nt = pchain.tile([128, KH, BS], F32, tag=f"nt{cid}")
                    nc.scalar.activation(out=nt[:], in_=st["np1"][:],
                                         func=AF.Tanh)
                    st["nt"] = nt

                def gate_out(cid, t, st):
                    eng = nc.vector if cid == 1 else nc.gpsimd
                    hsb = h1_sb if cid == 1 else h2_sb
                    prev = hsb[:, :, t * BS:(t + 1) * BS]
                    hm0 = pchain.tile([128, KH, BS], F32, tag=f"hm0{cid}")
                    eng.tensor_tensor(
                        out=hm0[:], in0=prev[:], in1=st["nt"][:],
                        op=AluOpType.subtract)
                    hm1 = pchain.tile([128, KH, BS], F32, tag=f"hm1{cid}")
                    eng.tensor_tensor(
                        out=hm1[:], in0=st["rzs"][:, KH:RZ, :], in1=hm0[:],
                        op=AluOpType.mult)
                    eng.tensor_tensor(
                        out=hsb[:, :, (t + 1) * BS:(t + 2) * BS],
                        in0=st["nt"][:], in1=hm1[:], op=AluOpType.add)

                def e3_block(t0, nsteps):
                    """gi2 for chain steps [t0, t0+nsteps); drains split S/V."""
                    n0 = t0 * BS
                    nn = nsteps * BS
                    enc = t0 < NF  # blocks never straddle NF
                    for m in range(MC):
                        ps = psum_mm.tile([128, 512], F32, tag="mm")
                        for k in range(KH):
                            nc.tensor.matmul(
                                ps[:, :nn],
                                lhsT=w2_sb[:, k, m * 128:(m + 1) * 128],
                                rhs=h1_sb[:, k, BS + n0:BS + n0 + nn],
                                start=(k == 0),
                                stop=(enc and k == KH - 1))
                        if not enc:
                            w0 = n0 - ROWS_E
                            for k in range(KW):
                                nc.tensor.matmul(
                                    ps[:, :nn],
                                    lhsT=w2_sb[:, KH + k, m * 128:(m + 1) * 128],
                                    rhs=words_sb[:, k, w0:w0 + nn],
                                    start=False, stop=(k == KW - 1))
                        # drains on Vector only: Scalar carries the chains'
                        # sigmoid/tanh critical path
                        nc.vector.tensor_scalar_add(
                            gi2_sb[:, m, n0:n0 + nn], ps[:, :nn],
                            bi2c_sb[:, m:m + 1])

                blocks = ([(t0, nn) for (t0, nn) in _ntiles(NF, 8)] +
                          [(NF + t0, nn) for (t0, nn) in _ntiles(TD, 9)])
                block_end = {t0 + nn: (t0, nn) for (t0, nn) in blocks}

                # Emission order per pair is chosen so each engine's in-order
                # stream never blocks on the other chain's not-yet-ready op:
                #   PE: h1 mms, h2 mms, (e3)
                #   S : sigr1 sigz1 tanh1 sigr2 sigz2 tanh2
                #   V : preloads(next) tmpn1 np0_1 np1_1 tmpn2 hm_1* np_2* hm_2*
                st1, st2 = {}, {}
                for tt in range(NSTEP + LAG):
                    t1 = tt if tt < NSTEP else None
                    s2 = tt - LAG if 0 <= tt - LAG < NSTEP else None
                    if t1 is not None:
                        chain_mms(1, t1)
                    if s2 is not None:
                        chain_mms(2, s2)
                    if t1 is not None:
                        gate_sig_r(1, t1, st1)
                        gate_np(1, t1, st1)
                        gate_tanh(1, st1)
                        gate_sig_z(1, t1, st1)
                        gate_out(1, t1, st1)
                        del gh_live[(1, t1)]
                    if s2 is not None:
                        gate_sig_r(2, s2, st2)
                        gate_np(2, s2, st2)
                        gate_tanh(2, st2)
                        gate_sig_z(2, s2, st2)
                        gate_out(2, s2, st2)
                        del gh_live[(2, s2)]
                    if t1 is not None and t1 + 1 in block_end:
                        e3_block(*block_end[t1 + 1])

            # ---- projection + log_softmax over decode steps ----
            # |logits| is bounded well below fp32 exp overflow here, so
            # log_softmax runs without the max shift: lp = x - ln(sum(e^x)).
            # Two row-tile groups: group 0's lse/subtract/output-DMA overlap
            # group 1's projection.  out_w and h2 in fp8 (pre-scaled).
            with (
                tc.tile_pool(name="pp", bufs=1) as pp,
                tc.tile_pool(name="pwst", bufs=3) as pwst,
                tc.tile_pool(name="pstage", bufs=3) as pstage,
                tc.tile_pool(name="psmall", bufs=2) as psmall,
            ):
                owT_r = owT[:].rearrange("(k p) n -> p k n", p=128)
                nvt = V // PN
                dcol0 = (NF + 1) * BS  # first decode h2 col
                mtiles = _ntiles(ROWS_D, 128)
                inv_s = 1.0 / (PSCALE * HSCALE)
                # h2 decode block, fp8 pre-scaled by HSCALE
                h2f8 = pp.tile([128, KH, ROWS_D], F8E4, tag="h2f8")
                for k in range(KH):
                    nc.vector.tensor_scalar_mul(
                        h2f8[:, k, :], h2_sb[:, k, dcol0:dcol0 + ROWS_D],
                        HSCALE)
                logits_t = [pp.tile([128, V], BF16, tag=f"logits{i}",
                                    name=f"logits{i}")
                            for i in range(len(mtiles))]
                sums_t = [psmall.tile([128, nvt], F32, tag=f"sums{i}",
                                      name=f"sums{i}")
                          for i in range(len(mtiles))]
                NCH = 4
                CW = V // NCH
                for g0 in range(0, len(mtiles), 2):
                    grp = [(mt, mtiles[mt])
                           for mt in range(g0, min(g0 + 2, len(mtiles)))]
                    for nt_i in range(nvt):
                        n0 = nt_i * PN
                        wst = pwst.tile([128, KH, PN], F8E4, tag="wst")
                        nc.sync.dma_start(out=wst[:], in_=owT_r[:, :, n0:n0 + PN])
                        for mt, (r0, mrows) in grp:
                            ps = psum_mm.tile([128, 512], F32, tag="mm")
                            last = KH // 2 - 1 if not cfg["has_out_b"] else None
                            for kp in range(KH // 2):
                                nc.tensor.matmul(
                                    ps[:mrows, :PN],
                                    lhsT=h2f8[:, 2 * kp:2 * kp + 2,
                                              r0:r0 + mrows],
                                    rhs=wst[:, 2 * kp:2 * kp + 2, :],
                                    start=(kp == 0), stop=(kp == last),
                                    perf_mode=mybir.MatmulPerfMode.DoubleRow)
                            if cfg["has_out_b"]:
                                nc.tensor.matmul(
                                    ps[:mrows, :PN],
                                    lhsT=ones_sb[:, :mrows],
                                    rhs=outb_sb[:, n0:n0 + PN],
                                    start=False, stop=True)
                            edump = pstage.tile([128, PN], BF16, tag="edump")
                            nc.scalar.activation(
                                out=edump[:mrows, :], in_=ps[:mrows, :PN],
                                func=AF.Exp, scale=inv_s,
                                accum_out=sums_t[mt][:mrows, nt_i:nt_i + 1])
                            nc.vector.tensor_scalar_mul(
                                logits_t[mt][:mrows, n0:n0 + PN],
                                ps[:mrows, :PN], inv_s)
                    # group tail: lse per row tile, then logp = logits - lse,
                    # alternating ACT / DVE per chunk; bf16 output via
                    # alternating gpsimd / sync DMA queues
                    for mt, (r0, mrows) in grp:
                        s1 = psmall.tile([128, 1], F32, tag="s1")
                        nc.vector.tensor_reduce(
                            out=s1[:mrows], in_=sums_t[mt][:mrows, :],
                            axis=mybir.AxisListType.X, op=AluOpType.add)
                        nshift = psmall.tile([128, 1], F32, tag="nshift")
                        nc.scalar.activation(
                            out=nshift[:mrows], in_=s1[:mrows], func=AF.Ln)
                        nc.vector.tensor_scalar_mul(
                            nshift[:mrows], nshift[:mrows], -1.0)
                        for c in range(NCH):
                            stage = pstage.tile([128, CW], BF16, tag="stage")
                            src = logits_t[mt][:mrows, c * CW:(c + 1) * CW]
                            if c % 2 == 0:
                                nc.scalar.activation(
                                    out=stage[:mrows, :], in_=src,
                                    func=AF.Identity, bias=nshift[:mrows])
                            else:
                                nc.vector.tensor_scalar_add(
                                    stage[:mrows, :], src, nshift[:mrows])
                            eng = nc.gpsimd if c % 2 == 0 else nc.sync
                            eng.dma_start(
                                out=out_slice(r0, mrows, c * CW, CW),
                                in_=stage[:mrows, :])
    return nc


# ---------------------------------------------------------------------------
# Host side
# ---------------------------------------------------------------------------
def _bf16(a):
    return np.ascontiguousarray(a, dtype=np.float32).astype(ml_dtypes.bfloat16)


def _f32(a):
    return np.ascontiguousarray(a, dtype=np.float32)


def _f8(a, scale):
    f8 = ml_dtypes.float8_e3m4
    f8max = float(ml_dtypes.finfo(f8).max)
    scaled = np.clip(np.asarray(a, dtype=np.float32) * scale, -f8max, f8max)
    return np.ascontiguousarray(scaled).astype(f8)


def _f8e4(a, scale):
    f8 = ml_dtypes.float8_e4m3fn
    f8max = float(ml_dtypes.finfo(f8).max)
    scaled = np.clip(np.asarray(a, dtype=np.float32) * scale, -f8max, f8max)
    return np.ascontiguousarray(scaled).astype(f8)


def prep_inputs(cfg, vid_feats, target_variable, emb, w_ih1, w_hh1, b_ih1,
                b_hh1, w_ih2, w_hh2, b_ih2, b_hh2, out_w, out_b):
    """Build per-core input maps."""
    BS, MC, KH, DH = cfg["BS"], cfg["MC"], cfg["KH"], cfg["DH"]
    TD, NC = cfg["TD"], cfg["n_cores"]

    vid_feats = np.asarray(vid_feats, dtype=np.float32)
    target_variable = np.asarray(target_variable)
    emb = np.asarray(emb, dtype=np.float32)

    shared = {
        "w1T": _bf16(np.asarray(w_ih1).T),
        "wh1T": _bf16(np.asarray(w_hh1).T),
        "w2T": _bf16(np.asarray(w_ih2).T),
        "wh2T": _bf16(np.asarray(w_hh2).T),
        "owT": _f8e4(np.asarray(out_w).T, PSCALE),
    }
    # combined biases: b_ih (+ b_hh for the r,z chunks; the n chunk of b_hh
    # is applied inside the gate, before the r multiply)
    def comb(bi, bh):
        c = np.asarray(bi, dtype=np.float32).copy()
        c[: 2 * DH] += np.asarray(bh, dtype=np.float32)[: 2 * DH]
        return c

    c1 = comb(b_ih1, b_hh1)
    c2 = comb(b_ih2, b_hh2)
    shared["bi1c"] = _f32(c1.reshape(MC, 128).T)
    shared["bi2c"] = _f32(c2.reshape(MC, 128).T)
    shared["gidec"] = _bf16(
        np.broadcast_to(c1.reshape(MC, 128).T[:, :, None], (128, MC, BS)))
    shared["ident"] = _bf16(np.eye(128))
    shared["bhnv1"] = _bf16(np.asarray(b_hh1, np.float32)[2 * DH:].reshape(1, DH))
    shared["bhnv2"] = _bf16(np.asarray(b_hh2, np.float32)[2 * DH:].reshape(1, DH))
    shared["onesb"] = _bf16(np.ones((1, BS)))
    if cfg["has_out_b"]:
        shared["outb"] = _f8(np.asarray(out_b).reshape(1, -1),
                             PSCALE * HSCALE)
        shared["ones"] = _f8(np.ones((1, 128)), 1.0)

    words = emb[np.asarray(target_variable[:, :TD], dtype=np.int64)]  # [B,TD,DW]

    in_maps = []
    for c in range(NC):
        sl = slice(c * BS, (c + 1) * BS)
        vs = vid_feats[sl]                      # [BS, NF, DV]
        ws = words[sl]                          # [BS, TD, DW]
        m = dict(shared)
        m["xT"] = _bf16(vs.transpose(2, 1, 0).reshape(cfg["DV"], -1))
        m["wordsT"] = _bf16(ws.transpose(2, 1, 0).reshape(cfg["DW"], -1))
        in_maps.append(m)
    return in_maps


_CACHE = {}
LAST_RESULT = None


def kernel(**inputs):
    global LAST_RESULT
    from concourse.bass_utils import run_bass_kernel_spmd

    DH = 512
    has_out_b = bool(np.any(np.asarray(inputs["out_b"])))
    c1 = np.asarray(inputs["b_ih1"], np.float32).copy()
    c1[:2 * DH] += np.asarray(inputs["b_hh1"], np.float32)[:2 * DH]
    has_bias1 = bool(np.any(c1))
    has_bhn1 = bool(np.any(np.asarray(inputs["b_hh1"])[2 * DH:]))
    has_bhn2 = bool(np.any(np.asarray(inputs["b_hh2"])[2 * DH:]))
    key = ("full", has_out_b, has_bias1, has_bhn1, has_bhn2)
    if key not in _CACHE:
        cfg = make_cfg(has_out_b=has_out_b, has_bias1=has_bias1,
                       has_bhn1=has_bhn1, has_bhn2=has_bhn2)
        _CACHE[key] = (cfg, build_nc(cfg))
    cfg, nc = _CACHE[key]

    in_maps = prep_inputs(cfg, **inputs)
    res = run_bass_kernel_spmd(nc, in_maps, list(range(cfg["n_cores"])))
    LAST_RESULT = res
    outs = [np.asarray(res.results[c]["out"]) for c in range(cfg["n_cores"])]
    return np.concatenate(outs, axis=0).astype(np.float32)  # [B, TD, V]


# revision 47
# speedup vs baseline: 1.0498x; 1.0006x over previous
"""Trainium2 Bass kernel for the caption-generation module (2-layer GRU
encoder-decoder + vocab projection + log_softmax).

Strategy: data-parallel over batch across 8 NeuronCores (B=128 -> 16 rows
per core, weights replicated).  Per core, transposed layout (feature dim on
SBUF partitions, (time*batch) on the free dim):

  E1:  gi1[t] = x_t @ w_ih1.T for all 40 encoder steps  (one batched matmul)
  C1/C2: h1/h2 chains, 67 sequential steps each, software-pipelined so the
       two chains' gate math interleaves op-by-op on Vector/Scalar while
       the PE runs the other chain's recurrent matmul.  gi (+b) for the
       r/z gates is preloaded into PSUM so the matmuls accumulate onto it
       and sigmoid reads PSUM directly.
  E3:  gi2[t] = [h1_t; w_t] @ w_ih2.T in blocks (batched matmul)
  P :  logits = h2_dec @ out_w.T (fp8), streamed log_softmax in two
       row-tile groups so the first group's output DMA overlaps the
       second group's projection; output written as bf16.
"""

import sys

sys.path.insert(0, "/opt/trn_rl_repo")

import numpy as np
import ml_dtypes

import concourse.bass as bass
import concourse.mybir as mybir
import concourse.tile as tile
from concourse.alu_op_type import AluOpType
from concourse.vector_clock import ScopedClock

# Align the Tile scheduler's PE cost model with measured hardware: a 16-col
# chain matmul sustains ~28ns/instruction on this device (issue-floor bound)
# vs ~7ns modeled at the 2.4GHz peak clock.  The skewed model makes the
# static scheduler bunch matmuls ahead of the gate ops and misorder the
# Scalar stream.  These constants are read lazily (once per process) by the
# rust cost model, so patch before the first build.  Schedule-order only —
# no effect on emitted semantics.
import concourse.hw_specs as _hw_specs

_hw_specs.TRN2Spec.PE_CYCLE = 1e9 / 0.6e9
_hw_specs.TRN2Spec.PE_CYCLE_PSTATE_MID = 1e9 / 0.55e9
_hw_specs.TRN2Spec.PE_CYCLE_PSTATE_LOW = 1e9 / 0.5e9

BF16 = mybir.dt.bfloat16
F32 = mybir.dt.float32
F8 = mybir.dt.float8e3
F8E4 = mybir.dt.float8e4
PSCALE = 64.0   # fp8 out_w pre-scale
HSCALE = 8.0    # fp8 h2 pre-scale
AF = mybir.ActivationFunctionType


# ---------------------------------------------------------------------------
# Workaround: this container's walrus rejects CTRL instructions carrying more
# than one sync-wait command.  Split the TileContext tail drain's wait list
# across a chain of drains, one wait each.
# ---------------------------------------------------------------------------
def _patched_drain_and_barrier(self, tick_clock, wait_clock):
    import bass_rust

    drain_inst = self.nc.sync.drain()
    wait_clock.add_sem_waits(
        drain_inst.ins, ScopedClock({None: tick_clock.global_clock})
    )
    waits = list(drain_inst.ins.sync_info.on_wait)
    if len(waits) > 1:
        si = drain_inst.ins.sync_info
        si.on_wait = waits[:1]
        drain_inst.ins.sync_info = si
        for i in range(1, len(waits)):
            extra = self.nc.sync.drain()
            extra.ins.sync_info = bass_rust.SyncInfo(
                on_wait=waits[i : i + 1], on_update=[]
            )
    self.nc.all_engine_barrier()
    assert self.sems is not None
    popped = self.nc._tile_sem_poison_stack.pop()
    assert popped is self._sem_poison
    self.nc.clear_and_free_semaphores(list(self.sems.allocated().values()))
    self.nc.all_engine_barrier()


tile.TileContext._drain_and_barrier = _patched_drain_and_barrier

# Same walrus limitation for regular engine instructions: at most one
# sync-wait per instruction.  Split extra waits onto preceding NoOps on the
# same engine (engine stalls there instead — identical semantics).
_orig_commit = tile.TileContext._commit_instruction


def _commit_split_waits(self, inst, lazy_reg_writes=True):
    si = getattr(inst, "sync_info", None)
    if (si is not None and si.on_wait and len(si.on_wait) > 1
            and inst.engine != mybir.EngineType.Unassigned):
        waits = list(si.on_wait)
        for w in waits[:-1]:
            nop = mybir.InstNoOp(
                name=self.nc.get_next_instruction_name(),
                sync_info=mybir.SyncInfo(on_wait=[w], on_update=[]),
                bass_nofuse=True,
                engine=inst.engine,
            )
            _orig_commit(self, nop, lazy_reg_writes=False)
        si.on_wait = waits[-1:]
        inst.sync_info = si
    return _orig_commit(self, inst, lazy_reg_writes)


tile.TileContext._commit_instruction = _commit_split_waits


# ---------------------------------------------------------------------------
# Config
# ---------------------------------------------------------------------------
def make_cfg(B=128, NF=40, TD=27, V=16000, DV=2048, DH=512, DW=512,
             n_cores=8, has_out_b=False, has_bias1=False, has_bhn1=False,
             has_bhn2=False):
    cfg = dict(B=B, NF=NF, TD=TD, V=V, DV=DV, DH=DH, DW=DW,
               n_cores=n_cores, has_out_b=has_out_b, has_bias1=has_bias1,
               has_bhn1=has_bhn1, has_bhn2=has_bhn2)
    cfg["BS"] = B // n_cores          # batch rows per core
    cfg["KV"] = DV // 128             # x feature chunks
    cfg["KH"] = DH // 128             # h feature chunks
    cfg["KW"] = DW // 128             # word feature chunks
    cfg["MC"] = 3 * DH // 128         # gate chunks
    cfg["NSTEP"] = NF + TD            # total chain steps
    cfg["ROWS_E"] = NF * cfg["BS"]    # encoder (t,b) columns
    cfg["ROWS_A"] = cfg["NSTEP"] * cfg["BS"]
    cfg["ROWS_D"] = TD * cfg["BS"]    # decode (t,b) columns
    # vocab tiling for the projection (psum free dim <= 512 fp32)
    for pn in (512, 500, 400, 320, 256):
        if V % pn == 0:
            cfg["PN"] = pn
            break
    else:
        raise ValueError(f"V={V} has no tile size")
    return cfg


def _ntiles(total, maxn):
    """Split `total` into tiles of at most maxn (last ragged)."""
    out = []
    n0 = 0
    while n0 < total:
        nn = min(maxn, total - n0)
        out.append((n0, nn))
        n0 += nn
    return out


# ---------------------------------------------------------------------------
# Kernel builder
# ---------------------------------------------------------------------------
def build_nc(cfg):
    BS, KV, KH, KW, MC = cfg["BS"], cfg["KV"], cfg["KH"], cfg["KW"], cfg["MC"]
    NF, TD, V, DH = cfg["NF"], cfg["TD"], cfg["V"], cfg["DH"]
    NSTEP, ROWS_E, ROWS_D = cfg["NSTEP"], cfg["ROWS_E"], cfg["ROWS_D"]
    PN = cfg["PN"]
    G3 = 3 * DH
    RZ = 2 * KH   # number of r+z gate chunks (psum-preloaded with gi)
    LAG = 12      # h2 chain trails h1 by this many steps (> E3 block size)

    nc = bass.Bass()

    # ---- DRAM parameters (per-core views; host prepares these) ----
    xT = nc.dram_tensor("xT", [cfg["DV"], ROWS_E], BF16, kind="ExternalInput")
    wordsT = nc.dram_tensor("wordsT", [cfg["DW"], ROWS_D], BF16, kind="ExternalInput")
    w1T = nc.dram_tensor("w1T", [cfg["DV"], G3], BF16, kind="ExternalInput")
    wh1T = nc.dram_tensor("wh1T", [DH, G3], BF16, kind="ExternalInput")
    w2T = nc.dram_tensor("w2T", [DH + cfg["DW"], G3], BF16, kind="ExternalInput")
    wh2T = nc.dram_tensor("wh2T", [DH, G3], BF16, kind="ExternalInput")
    owT = nc.dram_tensor("owT", [DH, V], F8E4, kind="ExternalInput")
    bi1c = nc.dram_tensor("bi1c", [128, MC], F32, kind="ExternalInput")
    bi2c = nc.dram_tensor("bi2c", [128, MC], F32, kind="ExternalInput")
    gidec = nc.dram_tensor("gidec", [128, MC, BS], BF16, kind="ExternalInput")
    ident = nc.dram_tensor("ident", [128, 128], BF16, kind="ExternalInput")
    bhnv1 = nc.dram_tensor("bhnv1", [1, DH], BF16, kind="ExternalInput")
    bhnv2 = nc.dram_tensor("bhnv2", [1, DH], BF16, kind="ExternalInput")
    onesb = nc.dram_tensor("onesb", [1, BS], BF16, kind="ExternalInput")
    if cfg["has_out_b"]:
        outb = nc.dram_tensor("outb", [1, V], F8, kind="ExternalInput")
        ones = nc.dram_tensor("ones", [1, 128], F8, kind="ExternalInput")
    out = nc.dram_tensor("out", [BS, TD, V], BF16, kind="ExternalOutput")
    # view [t, b, v] of out[b, t, v] (strides V, TD*V, 1); row r = t*BS + b
    _o = out[:]
    out_tbv = bass.AP(tensor=_o.tensor, offset=_o.offset,
                      ap=[[V, TD], [TD * V, BS], [1, V]])

    def out_slice(r0, mrows, c0, cw):
        assert r0 % BS == 0 and mrows % BS == 0
        return out_tbv[r0 // BS:(r0 + mrows) // BS, :, c0:c0 + cw]

    from contextlib import ExitStack

    with tile.TileContext(nc) as tc:
        with (
            tc.tile_pool(name="pconst", bufs=1) as pconst,
            tc.tile_pool(name="pchain", bufs=8) as pchain,
            tc.tile_pool(name="psum_mm", bufs=3, space="PSUM") as psum_mm,
            tc.tile_pool(name="psum_gh", bufs=5, space="PSUM") as psum_gh,
            tc.tile_pool(name="ph2", bufs=1) as ph2,
        ):
            # ---- constants ----
            bi1c_sb = pconst.tile([128, MC], F32, tag="bi1c")
            nc.sync.dma_start(out=bi1c_sb[:], in_=bi1c[:])
            bi2c_sb = pconst.tile([128, MC], F32, tag="bi2c")
            nc.sync.dma_start(out=bi2c_sb[:], in_=bi2c[:])
            gidec_sb = pconst.tile([128, MC, BS], BF16, tag="gidec")
            nc.sync.dma_start(out=gidec_sb[:], in_=gidec[:])
            ident_sb = pconst.tile([128, 128], BF16, tag="ident")
            nc.sync.dma_start(out=ident_sb[:], in_=ident[:])
            bhnv1_sb = pconst.tile([1, DH], BF16, tag="bhnv1")
            nc.sync.dma_start(out=bhnv1_sb[:], in_=bhnv1[:])
            bhnv2_sb = pconst.tile([1, DH], BF16, tag="bhnv2")
            nc.sync.dma_start(out=bhnv2_sb[:], in_=bhnv2[:])
            onesb_sb = pconst.tile([1, BS], BF16, tag="onesb")
            nc.sync.dma_start(out=onesb_sb[:], in_=onesb[:])
            if cfg["has_out_b"]:
                outb_sb = pconst.tile([1, V], F8, tag="outb")
                nc.sync.dma_start(out=outb_sb[:], in_=outb[:])
                ones_sb = pconst.tile([1, 128], F8, tag="ones")
                nc.sync.dma_start(out=ones_sb[:], in_=ones[:])

            h2_sb = ph2.tile([128, KH, (NSTEP + 1) * BS], BF16, tag="h2")
            nc.vector.memset(h2_sb[:, :, 0:BS], 0.0)

            # ====== E1, interleaved h1/E3/h2 chains ======
            with ExitStack() as chain_es:
                pmidA = chain_es.enter_context(tc.tile_pool(name="pmidA", bufs=1))
                h1_sb = pmidA.tile([128, KH, (NSTEP + 1) * BS], BF16, tag="h1")
                nc.vector.memset(h1_sb[:, :, 0:BS], 0.0)
                gi1_sb = pmidA.tile([128, MC, ROWS_E], BF16, tag="gi1")
                wh1_sb = pmidA.tile([128, KH, G3], BF16, tag="wh1")

                with tc.tile_pool(name="pw1", bufs=1) as pw1:
                    x_sb = pw1.tile([128, KV, ROWS_E], BF16, tag="x")
                    xT_r = xT[:].rearrange("(k p) n -> p k n", p=128)
                    for k in range(KV):
                        nc.sync.dma_start(out=x_sb[:, k, :], in_=xT_r[:, k, :])
                    w1_sb = pw1.tile([128, KV, G3], BF16, tag="w1")
                    w1T_r = w1T[:].rearrange("(k p) n -> p k n", p=128)
                    for k in range(KV):
                        nc.sync.dma_start(out=w1_sb[:, k, :], in_=w1T_r[:, k, :])
                    wh1T_r = wh1T[:].rearrange("(k p) n -> p k n", p=128)
                    for k in range(KH):
                        nc.sync.dma_start(out=wh1_sb[:, k, :], in_=wh1T_r[:, k, :])

                    # E1: gi1 = w1T.T @ x  (+ bias via ACT copy)
                    for (n0, nn) in _ntiles(ROWS_E, 320):
                        for m in range(MC):
                            ps = psum_mm.tile([128, 512], F32, tag="mm")
                            for k in range(KV):
                                nc.tensor.matmul(
                                    ps[:, :nn],
                                    lhsT=w1_sb[:, k, m * 128:(m + 1) * 128],
                                    rhs=x_sb[:, k, n0:n0 + nn],
                                    start=(k == 0), stop=(k == KV - 1))
                            nc.scalar.activation(
                                out=gi1_sb[:, m, n0:n0 + nn], in_=ps[:, :nn],
                                func=AF.Identity, bias=bi1c_sb[:, m:m + 1],
                                scale=1.0)

                # layer-2 weights / words / gi2 (loaded while chains run)
                pmidB = chain_es.enter_context(tc.tile_pool(name="pmidB", bufs=1))
                w2_sb = pmidB.tile([128, KH + KW, G3], BF16, tag="w2")
                w2T_r = w2T[:].rearrange("(k p) n -> p k n", p=128)
                for k in range(KH + KW):
                    nc.sync.dma_start(out=w2_sb[:, k, :], in_=w2T_r[:, k, :])
                words_sb = pmidB.tile([128, KW, ROWS_D], BF16, tag="words")
                wordsT_r = wordsT[:].rearrange("(k p) n -> p k n", p=128)
                for k in range(KW):
                    nc.sync.dma_start(out=words_sb[:, k, :], in_=wordsT_r[:, k, :])
                wh2_sb = pmidB.tile([128, KH, G3], BF16, tag="wh2")
                wh2T_r = wh2T[:].rearrange("(k p) n -> p k n", p=128)
                for k in range(KH):
                    nc.sync.dma_start(out=wh2_sb[:, k, :], in_=wh2T_r[:, k, :])
                gi2_sb = pmidB.tile([128, MC, NSTEP * BS], BF16, tag="gi2")

                # ---------------- chain machinery -------------------------
                # psum gh tile: one identity-matmul injects gi_rz (start=True
                # for the whole r/z region, so each m-region's accumulation
                # group closes at its own stop and consumers get fine-grained
                # waits: sigmoid fires right after the r-chunk matmuls).
                # m order: r chunks first, then n (np0), then z (hm1, last).
                M_ORDER = (list(range(KH)) + list(range(RZ, MC))
                           + list(range(KH, RZ)))
                gh_live = {}

                def girz_of(cid, t):
                    if cid == 1:
                        if t < NF:
                            return gi1_sb[:, 0:RZ, t * BS:(t + 1) * BS]
                        return gidec_sb[:, 0:RZ, :] if cfg["has_bias1"] else None
                    return gi2_sb[:, 0:RZ, t * BS:(t + 1) * BS]

                def gin_of(cid, t):
                    if cid == 1:
                        if t < NF:
                            return gi1_sb[:, RZ:, t * BS:(t + 1) * BS]
                        return gidec_sb[:, RZ:, :] if cfg["has_bias1"] else None
                    return gi2_sb[:, RZ:, t * BS:(t + 1) * BS]

                def chain_mms(cid, t):
                    gh = psum_gh.tile([128, MC, BS], F32, tag="gh")
                    gh_live[(cid, t)] = gh
                    whh = wh1_sb if cid == 1 else wh2_sb
                    hsb = h1_sb if cid == 1 else h2_sb
                    has_bhn = cfg["has_bhn1"] if cid == 1 else cfg["has_bhn2"]
                    bhnv_sb = bhnv1_sb if cid == 1 else bhnv2_sb
                    prev = hsb[:, :, t * BS:(t + 1) * BS]
                    inj = girz_of(cid, t)
                    if inj is not None:
                        nc.tensor.matmul(
                            gh[:, 0:RZ, :], lhsT=ident_sb[:], rhs=inj,
                            start=True, stop=False, skip_group_check=True)
                    for m in M_ORDER:
                        is_rz = m < RZ
                        if not is_rz and has_bhn:
                            # rank-1: gh_n += bhn_chunk (x) ones
                            nc.tensor.matmul(
                                gh[:, m, :],
                                lhsT=bhnv_sb[0:1, (m - RZ) * 128:(m - RZ + 1) * 128],
                                rhs=onesb_sb[0:1, :],
                                start=True, stop=False, skip_group_check=True)
                        for k in range(KH):
                            start = (k == 0) and (
                                (is_rz and inj is None)
                                or (not is_rz and not has_bhn))
                            nc.tensor.matmul(
                                gh[:, m, :],
                                lhsT=whh[:, k, m * 128:(m + 1) * 128],
                                rhs=prev[:, k, :],
                                start=start, stop=(k == KH - 1),
                                skip_group_check=True)

                def gate_sig_r(cid, t, st):
                    gh = gh_live[(cid, t)]
                    rzs = pchain.tile([128, RZ, BS], F32, tag=f"rzs{cid}")
                    nc.scalar.activation(out=rzs[:, 0:KH, :],
                                         in_=gh[:, 0:KH, :], func=AF.Sigmoid)
                    st["rzs"] = rzs

                def gate_sig_z(cid, t, st):
                    # emitted after tanh: z is needed only by hm1, so it must
                    # not sit between np0 and tanh in the Scalar stream
                    gh = gh_live[(cid, t)]
                    nc.scalar.activation(out=st["rzs"][:, KH:RZ, :],
                                         in_=gh[:, KH:RZ, :], func=AF.Sigmoid)

                def gate_np(cid, t, st):
                    # np0 reads the n-gate psum directly (includes bhn via the
                    # rank-1 matmul when nonzero); np1 adds gi_n when present.
                    # np0 reads PSUM so it stays on Vector for both chains;
                    # chain 2's remaining SBUF-only ops go to idle GpSimd.
                    gh = gh_live[(cid, t)]
                    eng1 = nc.vector
                    eng2 = nc.vector if cid == 1 else nc.gpsimd
                    np0 = pchain.tile([128, KH, BS], F32, tag=f"np0{cid}")
                    eng1.tensor_tensor(
                        out=np0[:], in0=st["rzs"][:, 0:KH, :],
                        in1=gh[:, RZ:, :], op=AluOpType.mult)
                    gin = gin_of(cid, t)
                    if gin is None:
                        st["np1"] = np0
                    else:
                        np1 = pchain.tile([128, KH, BS], F32, tag=f"np1{cid}")
                        eng2.tensor_tensor(
                            out=np1[:], in0=gin, in1=np0[:],
                            op=AluOpType.add)
                        st["np1"] = np1

                def gate_tanh(cid, st):
                    nt = pchain.tile([128, KH, BS], F32, tag=f"nt{cid}")
                    nc.scalar.activation(out=nt[:], in_=st["np1"][:],
                                         func=AF.Tanh)
                    st["nt"] = nt

                def gate_out(cid, t, st):
                    eng = nc.vector if cid == 1 else nc.gpsimd
                    hsb = h1_sb if cid == 1 else h2_sb
                    prev = hsb[:, :, t * BS:(t + 1) * BS]
                    hm0 = pchain.tile([128, KH, BS], F32, tag=f"hm0{cid}")
                    eng.tensor_tensor(
                        out=hm0[:], in0=prev[:], in1=st["nt"][:],
                        op=AluOpType.subtract)
                    hm1 = pchain.tile([128, KH, BS], F32, tag=f"hm1{cid}")
                    eng.tensor_tensor(
                        out=hm1[:], in0=st["rzs"][:, KH:RZ, :], in1=hm0[:],
                        op=AluOpType.mult)
                    eng.tensor_tensor(
                        out=hsb[:, :, (t + 1) * BS:(t + 2) * BS],
                        in0=st["nt"][:], in1=hm1[:], op=AluOpType.add)

                def e3_block(t0, nsteps):
                    """gi2 for chain steps [t0, t0+nsteps); drains split S/V."""
                    n0 = t0 * BS
                    nn = nsteps * BS
                    enc = t0 < NF  # blocks never straddle NF
                    for m in range(MC):
                        ps = psum_mm.tile([128, 512], F32, tag="mm")
                        for k in range(KH):
                            nc.tensor.matmul(
                                ps[:, :nn],
                                lhsT=w2_sb[:, k, m * 128:(m + 1) * 128],
                                rhs=h1_sb[:, k, BS + n0:BS + n0 + nn],
                                start=(k == 0),
                                stop=(enc and k == KH - 1))
                        if not enc:
                            w0 = n0 - ROWS_E
                            for k in range(KW):
                                nc.tensor.matmul(
                                    ps[:, :nn],
                                    lhsT=w2_sb[:, KH + k, m * 128:(m + 1) * 128],
                                    rhs=words_sb[:, k, w0:w0 + nn],
                                    start=False, stop=(k == KW - 1))
                        # drains on Vector only: Scalar carries the chains'
                        # sigmoid/tanh critical path
                        nc.vector.tensor_scalar_add(
                            gi2_sb[:, m, n0:n0 + nn], ps[:, :nn],
                            bi2c_sb[:, m:m + 1])

                blocks = ([(t0, nn) for (t0, nn) in _ntiles(NF, 8)] +
                          [(NF + t0, nn) for (t0, nn) in _ntiles(TD, 9)])
                block_end = {t0 + nn: (t0, nn) for (t0, nn) in blocks}

                # Emission order per pair is chosen so each engine's in-order
                # stream never blocks on the other chain's not-yet-ready op:
                #   PE: h1 mms, h2 mms, (e3)
                #   S : sigr1 sigz1 tanh1 sigr2 sigz2 tanh2
                #   V : preloads(next) tmpn1 np0_1 np1_1 tmpn2 hm_1* np_2* hm_2*
                st1, st2 = {}, {}
                for tt in range(NSTEP + LAG):
                    t1 = tt if tt < NSTEP else None
                    s2 = tt - LAG if 0 <= tt - LAG < NSTEP else None
                    if t1 is not None:
                        chain_mms(1, t1)
                    if s2 is not None:
                        chain_mms(2, s2)
                    if t1 is not None:
                        gate_sig_r(1, t1, st1)
                        gate_np(1, t1, st1)
                        gate_tanh(1, st1)
                        gate_sig_z(1, t1, st1)
                        gate_out(1, t1, st1)
                        del gh_live[(1, t1)]
                    if s2 is not None:
                        gate_sig_r(2, s2, st2)
                        gate_np(2, s2, st2)
                        gate_tanh(2, st2)
                        gate_sig_z(2, s2, st2)
                        gate_out(2, s2, st2)
                        del gh_live[(2, s2)]
                    if t1 is not None and t1 + 1 in block_end:
                        e3_block(*block_end[t1 + 1])

            # ---- projection + log_softmax over decode steps ----
            # |logits| is bounded well below fp32 exp overflow here, so
            # log_softmax runs without the max shift: lp = x - ln(sum(e^x)).
            # Two row-tile groups: group 0's lse/subtract/output-DMA overlap
            # group 1's projection.  out_w and h2 in fp8 (pre-scaled).
            with (
                tc.tile_pool(name="pp", bufs=1) as pp,
                tc.tile_pool(name="pwst", bufs=3) as pwst,
                tc.tile_pool(name="pstage", bufs=3) as pstage,
                tc.tile_pool(name="psmall", bufs=2) as psmall,
            ):
                owT_r = owT[:].rearrange("(k p) n -> p k n", p=128)
                nvt = V // PN
                dcol0 = (NF + 1) * BS  # first decode h2 col
                mtiles = _ntiles(ROWS_D, 128)
                inv_s = 1.0 / (PSCALE * HSCALE)
                # h2 decode block, fp8 pre-scaled by HSCALE
                h2f8 = pp.tile([128, KH, ROWS_D], F8E4, tag="h2f8")
                for k in range(KH):
                    nc.vector.tensor_scalar_mul(
                        h2f8[:, k, :], h2_sb[:, k, dcol0:dcol0 + ROWS_D],
                        HSCALE)
                logits_t = [pp.tile([128, V], BF16, tag=f"logits{i}",
                                    name=f"logits{i}")
                            for i in range(len(mtiles))]
                sums_t = [psmall.tile([128, nvt], F32, tag=f"sums{i}",
                                      name=f"sums{i}")
                          for i in range(len(mtiles))]
                NCH = 4
                CW = V // NCH
                for g0 in range(0, len(mtiles), 2):
                    grp = [(mt, mtiles[mt])
                           for mt in range(g0, min(g0 + 2, len(mtiles)))]
                    for nt_i in range(nvt):
                        n0 = nt_i * PN
                        wst = pwst.tile([128, KH, PN], F8E4, tag="wst")
                        nc.sync.dma_start(out=wst[:], in_=owT_r[:, :, n0:n0 + PN])
                        for mt, (r0, mrows) in grp:
                            ps = psum_mm.tile([128, 512], F32, tag="mm")
                            last = KH // 2 - 1 if not cfg["has_out_b"] else None
                            for kp in range(KH // 2):
                                nc.tensor.matmul(
                                    ps[:mrows, :PN],
                                    lhsT=h2f8[:, 2 * kp:2 * kp + 2,
                                              r0:r0 + mrows],
                                    rhs=wst[:, 2 * kp:2 * kp + 2, :],
                                    start=(kp == 0), stop=(kp == last),
                                    perf_mode=mybir.MatmulPerfMode.DoubleRow)
                            if cfg["has_out_b"]:
                                nc.tensor.matmul(
                                    ps[:mrows, :PN],
                                    lhsT=ones_sb[:, :mrows],
                                    rhs=outb_sb[:, n0:n0 + PN],
                                    start=False, stop=True)
                            edump = pstage.tile([128, PN], BF16, tag="edump")
                            nc.scalar.activation(
                                out=edump[:mrows, :], in_=ps[:mrows, :PN],
                                func=AF.Exp, scale=inv_s,
                                accum_out=sums_t[mt][:mrows, nt_i:nt_i + 1])
                            nc.vector.tensor_scalar_mul(
                                logits_t[mt][:mrows, n0:n0 + PN],
                                ps[:mrows, :PN], inv_s)
                    # group tail: lse per row tile, then logp = logits - lse,
                    # alternating ACT / DVE per chunk; bf16 output via
                    # alternating gpsimd / sync DMA queues
                    for mt, (r0, mrows) in grp:
                        s1 = psmall.tile([128, 1], F32, tag="s1")
                        nc.vector.tensor_reduce(
                            out=s1[:mrows], in_=sums_t[mt][:mrows, :],
                            axis=mybir.AxisListType.X, op=AluOpType.add)
                        nshift = psmall.tile([128, 1], F32, tag="nshift")
                        nc.scalar.activation(
                            out=nshift[:mrows], in_=s1[:mrows], func=AF.Ln)
                        nc.vector.tensor_scalar_mul(
                            nshift[:mrows], nshift[:mrows], -1.0)
                        for c in range(NCH):
                            stage = pstage.tile([128, CW], BF16, tag="stage")
                            src = logits_t[mt][:mrows, c * CW:(c + 1) * CW]
                            if c % 2 == 0:
                                nc.scalar.activation(
                                    out=stage[:mrows, :], in_=src,
                                    func=AF.Identity, bias=nshift[:mrows])
                            else:
                                nc.vector.tensor_scalar_add(
                                    stage[:mrows, :], src, nshift[:mrows])
                            eng = nc.gpsimd if c % 2 == 0 else nc.sync
                            eng.dma_start(
                                out=out_slice(r0, mrows, c * CW, CW),
                                in_=stage[:mrows, :])
    return nc


# ---------------------------------------------------------------------------
# Host side
# ---------------------------------------------------------------------------
def _bf16(a):
    return np.ascontiguousarray(a, dtype=np.float32).astype(ml_dtypes.bfloat16)


def _f32(a):
    return np.ascontiguousarray(a, dtype=np.float32)


def _f8(a, scale):
    f8 = ml_dtypes.float8_e3m4
    f8max = float(ml_dtypes.finfo(f8).max)
    scaled = np.clip(np.asarray(a, dtype=np.float32) * scale, -f8max, f8max)
    return np.ascontiguousarray(scaled).astype(f8)


def _f8e4(a, scale):
    f8 = ml_dtypes.float8_e4m3fn
    f8max = float(ml_dtypes.finfo(f8).max)
    scaled = np.clip(np.asarray(a, dtype=np.float32) * scale, -f8max, f8max)
    return np.ascontiguousarray(scaled).astype(f8)


def prep_inputs(cfg, vid_feats, target_variable, emb, w_ih1, w_hh1, b_ih1,
                b_hh1, w_ih2, w_hh2, b_ih2, b_hh2, out_w, out_b):
    """Build per-core input maps."""
    BS, MC, KH, DH = cfg["BS"], cfg["MC"], cfg["KH"], cfg["DH"]
    TD, NC = cfg["TD"], cfg["n_cores"]

    vid_feats = np.asarray(vid_feats, dtype=np.float32)
    target_variable = np.asarray(target_variable)
    emb = np.asarray(emb, dtype=np.float32)

    shared = {
        "w1T": _bf16(np.asarray(w_ih1).T),
        "wh1T": _bf16(np.asarray(w_hh1).T),
        "w2T": _bf16(np.asarray(w_ih2).T),
        "wh2T": _bf16(np.asarray(w_hh2).T),
        "owT": _f8e4(np.asarray(out_w).T, PSCALE),
    }
    # combined biases: b_ih (+ b_hh for the r,z chunks; the n chunk of b_hh
    # is applied inside the gate, before the r multiply)
    def comb(bi, bh):
        c = np.asarray(bi, dtype=np.float32).copy()
        c[: 2 * DH] += np.asarray(bh, dtype=np.float32)[: 2 * DH]
        return c

    c1 = comb(b_ih1, b_hh1)
    c2 = comb(b_ih2, b_hh2)
    shared["bi1c"] = _f32(c1.reshape(MC, 128).T)
    shared["bi2c"] = _f32(c2.reshape(MC, 128).T)
    shared["gidec"] = _bf16(
        np.broadcast_to(c1.reshape(MC, 128).T[:, :, None], (128, MC, BS)))
    shared["ident"] = _bf16(np.eye(128))
    shared["bhnv1"] = _bf16(np.asarray(b_hh1, np.float32)[2 * DH:].reshape(1, DH))
    shared["bhnv2"] = _bf16(np.asarray(b_hh2, np.float32)[2 * DH:].reshape(1, DH))
    shared["onesb"] = _bf16(np.ones((1, BS)))
    if cfg["has_out_b"]:
        shared["outb"] = _f8(np.asarray(out_b).reshape(1, -1),
                             PSCALE * HSCALE)
        shared["ones"] = _f8(np.ones((1, 128)), 1.0)

    words = emb[np.asarray(target_variable[:, :TD], dtype=np.int64)]  # [B,TD,DW]

    in_maps = []
    for c in range(NC):
        sl = slice(c * BS, (c + 1) * BS)
        vs = vid_feats[sl]                      # [BS, NF, DV]
        ws = words[sl]                          # [BS, TD, DW]
        m = dict(shared)
        m["xT"] = _bf16(vs.transpose(2, 1, 0).reshape(cfg["DV"], -1))
        m["wordsT"] = _bf16(ws.transpose(2, 1, 0).reshape(cfg["DW"], -1))
        in_maps.append(m)
    return in_maps


_CACHE = {}
LAST_RESULT = None


def kernel(**inputs):
    global LAST_RESULT
    from concourse.bass_utils import run_bass_kernel_spmd

    DH = 512
    has_out_b = bool(np.any(np.asarray(inputs["out_b"])))
    c1 = np.asarray(inputs["b_ih1"], np.float32).copy()
    c1[:2 * DH] += np.asarray(inputs["b_hh1"], np.float32)[:2 * DH]
    has_bias1 = bool(np.any(c1))
    has_bhn1 = bool(np.any(np.asarray(inputs["b_hh1"])[2 * DH:]))
    has_bhn2 = bool(np.any(np.asarray(inputs["b_hh2"])[2 * DH:]))
    key = ("full", has_out_b, has_bias1, has_bhn1, has_bhn2)
    if key not in _CACHE:
        cfg = make_cfg(has_out_b=has_out_b, has_bias1=has_bias1,
                       has_bhn1=has_bhn1, has_bhn2=has_bhn2)
        _CACHE[key] = (cfg, build_nc(cfg))
    cfg, nc = _CACHE[key]

    in_maps = prep_inputs(cfg, **inputs)
    res = run_bass_kernel_spmd(nc, in_maps, list(range(cfg["n_cores"])))
    LAST_RESULT = res
    outs = [np.asarray(res.results[c]["out"]) for c in range(cfg["n_cores"])]
    return np.concatenate(outs, axis=0).astype(np.float32)  # [B, TD, V]


# revision 48
# speedup vs baseline: 1.1033x; 1.0510x over previous
"""Trainium2 Bass kernel for the caption-generation module (2-layer GRU
encoder-decoder + vocab projection + log_softmax).

Strategy: data-parallel over batch across 8 NeuronCores (B=128 -> 16 rows
per core, weights replicated).  Per core, transposed layout (feature dim on
SBUF partitions, (time*batch) on the free dim):

  E1:  gi1[t] = x_t @ w_ih1.T for all 40 encoder steps  (one batched matmul)
  C1/C2: h1/h2 chains, 67 sequential steps each, software-pipelined so the
       two chains' gate math interleaves op-by-op on Vector/Scalar while
       the PE runs the other chain's recurrent matmul.  gi (+b) for the
       r/z gates is preloaded into PSUM so the matmuls accumulate onto it
       and sigmoid reads PSUM directly.
  E3:  gi2[t] = [h1_t; w_t] @ w_ih2.T in blocks (batched matmul)
  P :  logits = h2_dec @ out_w.T (fp8), streamed log_softmax in two
       row-tile groups so the first group's output DMA overlaps the
       second group's projection; output written as bf16.
"""

import sys

sys.path.insert(0, "/opt/trn_rl_repo")

import numpy as np
import ml_dtypes

import concourse.bass as bass
import concourse.mybir as mybir
import concourse.tile as tile
from concourse.alu_op_type import AluOpType
from concourse.vector_clock import ScopedClock

# Align the Tile scheduler's PE cost model with measured hardware: a 16-col
# chain matmul sustains ~28ns/instruction on this device (issue-floor bound)
# vs ~7ns modeled at the 2.4GHz peak clock.  The skewed model makes the
# static scheduler bunch matmuls ahead of the gate ops and misorder the
# Scalar stream.  These constants are read lazily (once per process) by the
# rust cost model, so patch before the first build.  Schedule-order only —
# no effect on emitted semantics.
import concourse.hw_specs as _hw_specs

_hw_specs.TRN2Spec.PE_CYCLE = 1e9 / 0.6e9
_hw_specs.TRN2Spec.PE_CYCLE_PSTATE_MID = 1e9 / 0.55e9
_hw_specs.TRN2Spec.PE_CYCLE_PSTATE_LOW = 1e9 / 0.5e9

BF16 = mybir.dt.bfloat16
F32 = mybir.dt.float32
F8 = mybir.dt.float8e3
F8E4 = mybir.dt.float8e4
PSCALE = 64.0   # fp8 out_w pre-scale
HSCALE = 8.0    # fp8 h2 pre-scale
AF = mybir.ActivationFunctionType


# ---------------------------------------------------------------------------
# Workaround: this container's walrus rejects CTRL instructions carrying more
# than one sync-wait command.  Split the TileContext tail drain's wait list
# across a chain of drains, one wait each.
# ---------------------------------------------------------------------------
def _patched_drain_and_barrier(self, tick_clock, wait_clock):
    import bass_rust

    drain_inst = self.nc.sync.drain()
    wait_clock.add_sem_waits(
        drain_inst.ins, ScopedClock({None: tick_clock.global_clock})
    )
    waits = list(drain_inst.ins.sync_info.on_wait)
    if len(waits) > 1:
        si = drain_inst.ins.sync_info
        si.on_wait = waits[:1]
        drain_inst.ins.sync_info = si
        for i in range(1, len(waits)):
            extra = self.nc.sync.drain()
            extra.ins.sync_info = bass_rust.SyncInfo(
                on_wait=waits[i : i + 1], on_update=[]
            )
    self.nc.all_engine_barrier()
    assert self.sems is not None
    popped = self.nc._tile_sem_poison_stack.pop()
    assert popped is self._sem_poison
    self.nc.clear_and_free_semaphores(list(self.sems.allocated().values()))
    self.nc.all_engine_barrier()


tile.TileContext._drain_and_barrier = _patched_drain_and_barrier

# Same walrus limitation for regular engine instructions: at most one
# sync-wait per instruction.  Split extra waits onto preceding NoOps on the
# same engine (engine stalls there instead — identical semantics).
_orig_commit = tile.TileContext._commit_instruction


def _commit_split_waits(self, inst, lazy_reg_writes=True):
    si = getattr(inst, "sync_info", None)
    if (si is not None and si.on_wait and len(si.on_wait) > 1
            and inst.engine != mybir.EngineType.Unassigned):
        waits = list(si.on_wait)
        for w in waits[:-1]:
            nop = mybir.InstNoOp(
                name=self.nc.get_next_instruction_name(),
                sync_info=mybir.SyncInfo(on_wait=[w], on_update=[]),
                bass_nofuse=True,
                engine=inst.engine,
            )
            _orig_commit(self, nop, lazy_reg_writes=False)
        si.on_wait = waits[-1:]
        inst.sync_info = si
    return _orig_commit(self, inst, lazy_reg_writes)


tile.TileContext._commit_instruction = _commit_split_waits


# ---------------------------------------------------------------------------
# Config
# ---------------------------------------------------------------------------
def make_cfg(B=128, NF=40, TD=27, V=16000, DV=2048, DH=512, DW=512,
             n_cores=8, has_out_b=False, has_bias1=False, has_bhn1=False,
             has_bhn2=False):
    cfg = dict(B=B, NF=NF, TD=TD, V=V, DV=DV, DH=DH, DW=DW,
               n_cores=n_cores, has_out_b=has_out_b, has_bias1=has_bias1,
               has_bhn1=has_bhn1, has_bhn2=has_bhn2)
    cfg["BS"] = B // n_cores          # batch rows per core
    cfg["KV"] = DV // 128             # x feature chunks
    cfg["KH"] = DH // 128             # h feature chunks
    cfg["KW"] = DW // 128             # word feature chunks
    cfg["MC"] = 3 * DH // 128         # gate chunks
    cfg["NSTEP"] = NF + TD            # total chain steps
    cfg["ROWS_E"] = NF * cfg["BS"]    # encoder (t,b) columns
    cfg["ROWS_A"] = cfg["NSTEP"] * cfg["BS"]
    cfg["ROWS_D"] = TD * cfg["BS"]    # decode (t,b) columns
    # vocab tiling for the projection (psum free dim <= 512 fp32)
    for pn in (512, 500, 400, 320, 256):
        if V % pn == 0:
            cfg["PN"] = pn
            break
    else:
        raise ValueError(f"V={V} has no tile size")
    return cfg


def _ntiles(total, maxn):
    """Split `total` into tiles of at most maxn (last ragged)."""
    out = []
    n0 = 0
    while n0 < total:
        nn = min(maxn, total - n0)
        out.append((n0, nn))
        n0 += nn
    return out


# ---------------------------------------------------------------------------
# Kernel builder
# ---------------------------------------------------------------------------
def build_nc(cfg):
    BS, KV, KH, KW, MC = cfg["BS"], cfg["KV"], cfg["KH"], cfg["KW"], cfg["MC"]
    NF, TD, V, DH = cfg["NF"], cfg["TD"], cfg["V"], cfg["DH"]
    NSTEP, ROWS_E, ROWS_D = cfg["NSTEP"], cfg["ROWS_E"], cfg["ROWS_D"]
    PN = cfg["PN"]
    G3 = 3 * DH
    RZ = 2 * KH   # number of r+z gate chunks (psum-preloaded with gi)
    LAG = 12      # h2 chain trails h1 by this many steps (> E3 block size)

    nc = bass.Bass()

    # ---- DRAM parameters (per-core views; host prepares these) ----
    xT = nc.dram_tensor("xT", [cfg["DV"], ROWS_E], F8E4, kind="ExternalInput")
    wordsT = nc.dram_tensor("wordsT", [cfg["DW"], ROWS_D], BF16, kind="ExternalInput")
    w1T = nc.dram_tensor("w1T", [cfg["DV"], G3], F8E4, kind="ExternalInput")
    wh1T = nc.dram_tensor("wh1T", [DH, G3], BF16, kind="ExternalInput")
    w2T = nc.dram_tensor("w2T", [DH + cfg["DW"], G3], BF16, kind="ExternalInput")
    wh2T = nc.dram_tensor("wh2T", [DH, G3], BF16, kind="ExternalInput")
    owT = nc.dram_tensor("owT", [DH, V], F8E4, kind="ExternalInput")
    bi1c = nc.dram_tensor("bi1c", [128, MC], F32, kind="ExternalInput")
    bi2c = nc.dram_tensor("bi2c", [128, MC], F32, kind="ExternalInput")
    gidec = nc.dram_tensor("gidec", [128, MC, BS], BF16, kind="ExternalInput")
    ident = nc.dram_tensor("ident", [128, 128], BF16, kind="ExternalInput")
    bhnv1 = nc.dram_tensor("bhnv1", [1, DH], BF16, kind="ExternalInput")
    bhnv2 = nc.dram_tensor("bhnv2", [1, DH], BF16, kind="ExternalInput")
    onesb = nc.dram_tensor("onesb", [1, BS], BF16, kind="ExternalInput")
    if cfg["has_out_b"]:
        outb = nc.dram_tensor("outb", [1, V], F8, kind="ExternalInput")
        ones = nc.dram_tensor("ones", [1, 128], F8, kind="ExternalInput")
    out = nc.dram_tensor("out", [BS, TD, V], BF16, kind="ExternalOutput")
    # view [t, b, v] of out[b, t, v] (strides V, TD*V, 1); row r = t*BS + b
    _o = out[:]
    out_tbv = bass.AP(tensor=_o.tensor, offset=_o.offset,
                      ap=[[V, TD], [TD * V, BS], [1, V]])

    def out_slice(r0, mrows, c0, cw):
        assert r0 % BS == 0 and mrows % BS == 0
        return out_tbv[r0 // BS:(r0 + mrows) // BS, :, c0:c0 + cw]

    from contextlib import ExitStack

    with tile.TileContext(nc) as tc:
        with (
            tc.tile_pool(name="pconst", bufs=1) as pconst,
            tc.tile_pool(name="pchain", bufs=8) as pchain,
            tc.tile_pool(name="psum_mm", bufs=3, space="PSUM") as psum_mm,
            tc.tile_pool(name="psum_gh", bufs=5, space="PSUM") as psum_gh,
            tc.tile_pool(name="ph2", bufs=1) as ph2,
        ):
            # ---- constants ----
            bi1c_sb = pconst.tile([128, MC], F32, tag="bi1c")
            nc.sync.dma_start(out=bi1c_sb[:], in_=bi1c[:])
            bi2c_sb = pconst.tile([128, MC], F32, tag="bi2c")
            nc.sync.dma_start(out=bi2c_sb[:], in_=bi2c[:])
            gidec_sb = pconst.tile([128, MC, BS], BF16, tag="gidec")
            nc.sync.dma_start(out=gidec_sb[:], in_=gidec[:])
            ident_sb = pconst.tile([128, 128], BF16, tag="ident")
            nc.sync.dma_start(out=ident_sb[:], in_=ident[:])
            bhnv1_sb = pconst.tile([1, DH], BF16, tag="bhnv1")
            nc.sync.dma_start(out=bhnv1_sb[:], in_=bhnv1[:])
            bhnv2_sb = pconst.tile([1, DH], BF16, tag="bhnv2")
            nc.sync.dma_start(out=bhnv2_sb[:], in_=bhnv2[:])
            onesb_sb = pconst.tile([1, BS], BF16, tag="onesb")
            nc.sync.dma_start(out=onesb_sb[:], in_=onesb[:])
            if cfg["has_out_b"]:
                outb_sb = pconst.tile([1, V], F8, tag="outb")
                nc.sync.dma_start(out=outb_sb[:], in_=outb[:])
                ones_sb = pconst.tile([1, 128], F8, tag="ones")
                nc.sync.dma_start(out=ones_sb[:], in_=ones[:])

            h2_sb = ph2.tile([128, KH, (NSTEP + 1) * BS], BF16, tag="h2")
            nc.vector.memset(h2_sb[:, :, 0:BS], 0.0)

            # ====== E1, interleaved h1/E3/h2 chains ======
            with ExitStack() as chain_es:
                pmidA = chain_es.enter_context(tc.tile_pool(name="pmidA", bufs=1))
                h1_sb = pmidA.tile([128, KH, (NSTEP + 1) * BS], BF16, tag="h1")
                nc.vector.memset(h1_sb[:, :, 0:BS], 0.0)
                gi1_sb = pmidA.tile([128, MC, ROWS_E], BF16, tag="gi1")
                wh1_sb = pmidA.tile([128, KH, G3], BF16, tag="wh1")

                with tc.tile_pool(name="pw1", bufs=1) as pw1:
                    x_sb = pw1.tile([128, KV, ROWS_E], F8E4, tag="x")
                    xT_r = xT[:].rearrange("(k p) n -> p k n", p=128)
                    for k in range(KV):
                        nc.sync.dma_start(out=x_sb[:, k, :], in_=xT_r[:, k, :])
                    w1_sb = pw1.tile([128, KV, G3], F8E4, tag="w1")
                    w1T_r = w1T[:].rearrange("(k p) n -> p k n", p=128)
                    for k in range(KV):
                        nc.sync.dma_start(out=w1_sb[:, k, :], in_=w1T_r[:, k, :])
                    wh1T_r = wh1T[:].rearrange("(k p) n -> p k n", p=128)
                    for k in range(KH):
                        nc.sync.dma_start(out=wh1_sb[:, k, :], in_=wh1T_r[:, k, :])

                    # E1: gi1 = w1T.T @ x  (+ bias via ACT copy)
                    for (n0, nn) in _ntiles(ROWS_E, 320):
                        for m in range(MC):
                            ps = psum_mm.tile([128, 512], F32, tag="mm")
                            for kp in range(KV // 2):
                                nc.tensor.matmul(
                                    ps[:, :nn],
                                    lhsT=w1_sb[:, 2 * kp:2 * kp + 2,
                                               m * 128:(m + 1) * 128],
                                    rhs=x_sb[:, 2 * kp:2 * kp + 2,
                                             n0:n0 + nn],
                                    start=(kp == 0), stop=(kp == KV // 2 - 1),
                                    perf_mode=mybir.MatmulPerfMode.DoubleRow)
                            nc.scalar.activation(
                                out=gi1_sb[:, m, n0:n0 + nn], in_=ps[:, :nn],
                                func=AF.Identity, bias=bi1c_sb[:, m:m + 1],
                                scale=1.0 / (8.0 * 256.0))

                # layer-2 weights / words / gi2 (loaded while chains run)
                pmidB = chain_es.enter_context(tc.tile_pool(name="pmidB", bufs=1))
                w2_sb = pmidB.tile([128, KH + KW, G3], BF16, tag="w2")
                w2T_r = w2T[:].rearrange("(k p) n -> p k n", p=128)
                for k in range(KH + KW):
                    nc.sync.dma_start(out=w2_sb[:, k, :], in_=w2T_r[:, k, :])
                words_sb = pmidB.tile([128, KW, ROWS_D], BF16, tag="words")
                wordsT_r = wordsT[:].rearrange("(k p) n -> p k n", p=128)
                for k in range(KW):
                    nc.sync.dma_start(out=words_sb[:, k, :], in_=wordsT_r[:, k, :])
                wh2_sb = pmidB.tile([128, KH, G3], BF16, tag="wh2")
                wh2T_r = wh2T[:].rearrange("(k p) n -> p k n", p=128)
                for k in range(KH):
                    nc.sync.dma_start(out=wh2_sb[:, k, :], in_=wh2T_r[:, k, :])
                gi2_sb = pmidB.tile([128, MC, NSTEP * BS], BF16, tag="gi2")

                # ---------------- chain machinery -------------------------
                # psum gh tile: one identity-matmul injects gi_rz (start=True
                # for the whole r/z region, so each m-region's accumulation
                # group closes at its own stop and consumers get fine-grained
                # waits: sigmoid fires right after the r-chunk matmuls).
                # m order: r chunks first, then n (np0), then z (hm1, last).
                M_ORDER = (list(range(KH)) + list(range(RZ, MC))
                           + list(range(KH, RZ)))
                gh_live = {}

                def girz_of(cid, t):
                    if cid == 1:
                        if t < NF:
                            return gi1_sb[:, 0:RZ, t * BS:(t + 1) * BS]
                        return gidec_sb[:, 0:RZ, :] if cfg["has_bias1"] else None
                    return gi2_sb[:, 0:RZ, t * BS:(t + 1) * BS]

                def gin_of(cid, t):
                    if cid == 1:
                        if t < NF:
                            return gi1_sb[:, RZ:, t * BS:(t + 1) * BS]
                        return gidec_sb[:, RZ:, :] if cfg["has_bias1"] else None
                    return gi2_sb[:, RZ:, t * BS:(t + 1) * BS]

                def chain_mms(cid, t):
                    gh = psum_gh.tile([128, MC, BS], F32, tag="gh")
                    gh_live[(cid, t)] = gh
                    whh = wh1_sb if cid == 1 else wh2_sb
                    hsb = h1_sb if cid == 1 else h2_sb
                    has_bhn = cfg["has_bhn1"] if cid == 1 else cfg["has_bhn2"]
                    bhnv_sb = bhnv1_sb if cid == 1 else bhnv2_sb
                    prev = hsb[:, :, t * BS:(t + 1) * BS]
                    inj = girz_of(cid, t)
                    if inj is not None:
                        nc.tensor.matmul(
                            gh[:, 0:RZ, :], lhsT=ident_sb[:], rhs=inj,
                            start=True, stop=False, skip_group_check=True)
                    for m in M_ORDER:
                        is_rz = m < RZ
                        if not is_rz and has_bhn:
                            # rank-1: gh_n += bhn_chunk (x) ones
                            nc.tensor.matmul(
                                gh[:, m, :],
                                lhsT=bhnv_sb[0:1, (m - RZ) * 128:(m - RZ + 1) * 128],
                                rhs=onesb_sb[0:1, :],
                                start=True, stop=False, skip_group_check=True)
                        for k in range(KH):
                            start = (k == 0) and (
                                (is_rz and inj is None)
                                or (not is_rz and not has_bhn))
                            nc.tensor.matmul(
                                gh[:, m, :],
                                lhsT=whh[:, k, m * 128:(m + 1) * 128],
                                rhs=prev[:, k, :],
                                start=start, stop=(k == KH - 1),
                                skip_group_check=True)

                def gate_sig_r(cid, t, st):
                    gh = gh_live[(cid, t)]
                    rzs = pchain.tile([128, RZ, BS], F32, tag=f"rzs{cid}")
                    nc.scalar.activation(out=rzs[:, 0:KH, :],
                                         in_=gh[:, 0:KH, :], func=AF.Sigmoid)
                    st["rzs"] = rzs

                def gate_sig_z(cid, t, st):
                    # emitted after tanh: z is needed only by hm1, so it must
                    # not sit between np0 and tanh in the Scalar stream
                    gh = gh_live[(cid, t)]
                    nc.scalar.activation(out=st["rzs"][:, KH:RZ, :],
                                         in_=gh[:, KH:RZ, :], func=AF.Sigmoid)

                def gate_np(cid, t, st):
                    # np0 reads the n-gate psum directly (includes bhn via the
                    # rank-1 matmul when nonzero); np1 adds gi_n when present.
                    # np0 reads PSUM so it stays on Vector for both chains;
                    # chain 2's remaining SBUF-only ops go to idle GpSimd.
                    gh = gh_live[(cid, t)]
                    eng1 = nc.vector
                    eng2 = nc.vector if cid == 1 else nc.gpsimd
                    np0 = pchain.tile([128, KH, BS], F32, tag=f"np0{cid}")
                    eng1.tensor_tensor(
                        out=np0[:], in0=st["rzs"][:, 0:KH, :],
                        in1=gh[:, RZ:, :], op=AluOpType.mult)
                    gin = gin_of(cid, t)
                    if gin is None:
                        st["np1"] = np0
                    else:
                        np1 = pchain.tile([128, KH, BS], F32, tag=f"np1{cid}")
                        eng2.tensor_tensor(
                            out=np1[:], in0=gin, in1=np0[:],
                            op=AluOpType.add)
                        st["np1"] = np1

                def gate_tanh(cid, st):
                    nt = pchain.tile([128, KH, BS], F32, tag=f"nt{cid}")
                    nc.scalar.activation(out=nt[:], in_=st["np1"][:],
                                         func=AF.Tanh)
                    st["nt"] = nt

                def gate_out(cid, t, st):
                    eng = nc.vector if cid == 1 else nc.gpsimd
                    hsb = h1_sb if cid == 1 else h2_sb
                    prev = hsb[:, :, t * BS:(t + 1) * BS]
                    hm0 = pchain.tile([128, KH, BS], F32, tag=f"hm0{cid}")
                    eng.tensor_tensor(
                        out=hm0[:], in0=prev[:], in1=st["nt"][:],
                        op=AluOpType.subtract)
                    hm1 = pchain.tile([128, KH, BS], F32, tag=f"hm1{cid}")
                    eng.tensor_tensor(
                        out=hm1[:], in0=st["rzs"][:, KH:RZ, :], in1=hm0[:],
                        op=AluOpType.mult)
                    eng.tensor_tensor(
                        out=hsb[:, :, (t + 1) * BS:(t + 2) * BS],
                        in0=st["nt"][:], in1=hm1[:], op=AluOpType.add)

                def e3_block(t0, nsteps):
                    """gi2 for chain steps [t0, t0+nsteps); drains split S/V."""
                    n0 = t0 * BS
                    nn = nsteps * BS
                    enc = t0 < NF  # blocks never straddle NF
                    for m in range(MC):
                        ps = psum_mm.tile([128, 512], F32, tag="mm")
                        for k in range(KH):
                            nc.tensor.matmul(
                                ps[:, :nn],
                                lhsT=w2_sb[:, k, m * 128:(m + 1) * 128],
                                rhs=h1_sb[:, k, BS + n0:BS + n0 + nn],
                                start=(k == 0),
                                stop=(enc and k == KH - 1))
                        if not enc:
                            w0 = n0 - ROWS_E
                            for k in range(KW):
                                nc.tensor.matmul(
                                    ps[:, :nn],
                                    lhsT=w2_sb[:, KH + k, m * 128:(m + 1) * 128],
                                    rhs=words_sb[:, k, w0:w0 + nn],
                                    start=False, stop=(k == KW - 1))
                        # drains on Vector only: Scalar carries the chains'
                        # sigmoid/tanh critical path
                        nc.vector.tensor_scalar_add(
                            gi2_sb[:, m, n0:n0 + nn], ps[:, :nn],
                            bi2c_sb[:, m:m + 1])

                blocks = ([(t0, nn) for (t0, nn) in _ntiles(NF, 8)] +
                          [(NF + t0, nn) for (t0, nn) in _ntiles(TD, 9)])
                block_end = {t0 + nn: (t0, nn) for (t0, nn) in blocks}

                # Emission order per pair is chosen so each engine's in-order
                # stream never blocks on the other chain's not-yet-ready op:
                #   PE: h1 mms, h2 mms, (e3)
                #   S : sigr1 sigz1 tanh1 sigr2 sigz2 tanh2
                #   V : preloads(next) tmpn1 np0_1 np1_1 tmpn2 hm_1* np_2* hm_2*
                st1, st2 = {}, {}
                for tt in range(NSTEP + LAG):
                    t1 = tt if tt < NSTEP else None
                    s2 = tt - LAG if 0 <= tt - LAG < NSTEP else None
                    if t1 is not None:
                        chain_mms(1, t1)
                    if s2 is not None:
                        chain_mms(2, s2)
                    if t1 is not None:
                        gate_sig_r(1, t1, st1)
                        gate_np(1, t1, st1)
                        gate_tanh(1, st1)
                        gate_sig_z(1, t1, st1)
                        gate_out(1, t1, st1)
                        del gh_live[(1, t1)]
                    if s2 is not None:
                        gate_sig_r(2, s2, st2)
                        gate_np(2, s2, st2)
                        gate_tanh(2, st2)
                        gate_sig_z(2, s2, st2)
                        gate_out(2, s2, st2)
                        del gh_live[(2, s2)]
                    if t1 is not None and t1 + 1 in block_end:
                        e3_block(*block_end[t1 + 1])

            # ---- projection + log_softmax over decode steps ----
            # |logits| is bounded well below fp32 exp overflow here, so
            # log_softmax runs without the max shift: lp = x - ln(sum(e^x)).
            # Two row-tile groups: group 0's lse/subtract/output-DMA overlap
            # group 1's projection.  out_w and h2 in fp8 (pre-scaled).
            with (
                tc.tile_pool(name="pp", bufs=1) as pp,
                tc.tile_pool(name="pwst", bufs=3) as pwst,
                tc.tile_pool(name="pstage", bufs=3) as pstage,
                tc.tile_pool(name="psmall", bufs=2) as psmall,
            ):
                owT_r = owT[:].rearrange("(k p) n -> p k n", p=128)
                nvt = V // PN
                dcol0 = (NF + 1) * BS  # first decode h2 col
                mtiles = _ntiles(ROWS_D, 128)
                inv_s = 1.0 / (PSCALE * HSCALE)
                # h2 decode block, fp8 pre-scaled by HSCALE
                h2f8 = pp.tile([128, KH, ROWS_D], F8E4, tag="h2f8")
                for k in range(KH):
                    nc.vector.tensor_scalar_mul(
                        h2f8[:, k, :], h2_sb[:, k, dcol0:dcol0 + ROWS_D],
                        HSCALE)
                logits_t = [pp.tile([128, V], BF16, tag=f"logits{i}",
                                    name=f"logits{i}")
                            for i in range(len(mtiles))]
                sums_t = [psmall.tile([128, nvt], F32, tag=f"sums{i}",
                                      name=f"sums{i}")
                          for i in range(len(mtiles))]
                NCH = 4
                CW = V // NCH
                for g0 in range(0, len(mtiles), 2):
                    grp = [(mt, mtiles[mt])
                           for mt in range(g0, min(g0 + 2, len(mtiles)))]
                    for nt_i in range(nvt):
                        n0 = nt_i * PN
                        wst = pwst.tile([128, KH, PN], F8E4, tag="wst")
                        nc.sync.dma_start(out=wst[:], in_=owT_r[:, :, n0:n0 + PN])
                        for mt, (r0, mrows) in grp:
                            ps = psum_mm.tile([128, 512], F32, tag="mm")
                            last = KH // 2 - 1 if not cfg["has_out_b"] else None
                            for kp in range(KH // 2):
                                nc.tensor.matmul(
                                    ps[:mrows, :PN],
                                    lhsT=h2f8[:, 2 * kp:2 * kp + 2,
                                              r0:r0 + mrows],
                                    rhs=wst[:, 2 * kp:2 * kp + 2, :],
                                    start=(kp == 0), stop=(kp == last),
                                    perf_mode=mybir.MatmulPerfMode.DoubleRow)
                            if cfg["has_out_b"]:
                                nc.tensor.matmul(
                                    ps[:mrows, :PN],
                                    lhsT=ones_sb[:, :mrows],
                                    rhs=outb_sb[:, n0:n0 + PN],
                                    start=False, stop=True)
                            edump = pstage.tile([128, PN], BF16, tag="edump")
                            nc.scalar.activation(
                                out=edump[:mrows, :], in_=ps[:mrows, :PN],
                                func=AF.Exp, scale=inv_s,
                                accum_out=sums_t[mt][:mrows, nt_i:nt_i + 1])
                            nc.vector.tensor_scalar_mul(
                                logits_t[mt][:mrows, n0:n0 + PN],
                                ps[:mrows, :PN], inv_s)
                    # group tail: lse per row tile, then logp = logits - lse,
                    # alternating ACT / DVE per chunk; bf16 output via
                    # alternating gpsimd / sync DMA queues
                    for mt, (r0, mrows) in grp:
                        s1 = psmall.tile([128, 1], F32, tag="s1")
                        nc.vector.tensor_reduce(
                            out=s1[:mrows], in_=sums_t[mt][:mrows, :],
                            axis=mybir.AxisListType.X, op=AluOpType.add)
                        nshift = psmall.tile([128, 1], F32, tag="nshift")
                        nc.scalar.activation(
                            out=nshift[:mrows], in_=s1[:mrows], func=AF.Ln)
                        nc.vector.tensor_scalar_mul(
                            nshift[:mrows], nshift[:mrows], -1.0)
                        for c in range(NCH):
                            stage = pstage.tile([128, CW], BF16, tag="stage")
                            src = logits_t[mt][:mrows, c * CW:(c + 1) * CW]
                            if c % 2 == 0:
                                nc.scalar.activation(
                                    out=stage[:mrows, :], in_=src,
                                    func=AF.Identity, bias=nshift[:mrows])
                            else:
                                nc.vector.tensor_scalar_add(
                                    stage[:mrows, :], src, nshift[:mrows])
                            eng = nc.gpsimd if c % 2 == 0 else nc.sync
                            eng.dma_start(
                                out=out_slice(r0, mrows, c * CW, CW),
                                in_=stage[:mrows, :])
    return nc


# ---------------------------------------------------------------------------
# Host side
# ---------------------------------------------------------------------------
def _bf16(a):
    return np.ascontiguousarray(a, dtype=np.float32).astype(ml_dtypes.bfloat16)


def _f32(a):
    return np.ascontiguousarray(a, dtype=np.float32)


def _f8(a, scale):
    f8 = ml_dtypes.float8_e3m4
    f8max = float(ml_dtypes.finfo(f8).max)
    scaled = np.clip(np.asarray(a, dtype=np.float32) * scale, -f8max, f8max)
    return np.ascontiguousarray(scaled).astype(f8)


def _f8e4(a, scale):
    f8 = ml_dtypes.float8_e4m3fn
    f8max = float(ml_dtypes.finfo(f8).max)
    scaled = np.clip(np.asarray(a, dtype=np.float32) * scale, -f8max, f8max)
    return np.ascontiguousarray(scaled).astype(f8)


def prep_inputs(cfg, vid_feats, target_variable, emb, w_ih1, w_hh1, b_ih1,
                b_hh1, w_ih2, w_hh2, b_ih2, b_hh2, out_w, out_b):
    """Build per-core input maps."""
    BS, MC, KH, DH = cfg["BS"], cfg["MC"], cfg["KH"], cfg["DH"]
    TD, NC = cfg["TD"], cfg["n_cores"]

    vid_feats = np.asarray(vid_feats, dtype=np.float32)
    target_variable = np.asarray(target_variable)
    emb = np.asarray(emb, dtype=np.float32)

    shared = {
        "w1T": _f8e4(np.asarray(w_ih1).T, 256.0),
        "wh1T": _bf16(np.asarray(w_hh1).T),
        "w2T": _bf16(np.asarray(w_ih2).T),
        "wh2T": _bf16(np.asarray(w_hh2).T),
        "owT": _f8e4(np.asarray(out_w).T, PSCALE),
    }
    # combined biases: b_ih (+ b_hh for the r,z chunks; the n chunk of b_hh
    # is applied inside the gate, before the r multiply)
    def comb(bi, bh):
        c = np.asarray(bi, dtype=np.float32).copy()
        c[: 2 * DH] += np.asarray(bh, dtype=np.float32)[: 2 * DH]
        return c

    c1 = comb(b_ih1, b_hh1)
    c2 = comb(b_ih2, b_hh2)
    shared["bi1c"] = _f32(c1.reshape(MC, 128).T)
    shared["bi2c"] = _f32(c2.reshape(MC, 128).T)
    shared["gidec"] = _bf16(
        np.broadcast_to(c1.reshape(MC, 128).T[:, :, None], (128, MC, BS)))
    shared["ident"] = _bf16(np.eye(128))
    shared["bhnv1"] = _bf16(np.asarray(b_hh1, np.float32)[2 * DH:].reshape(1, DH))
    shared["bhnv2"] = _bf16(np.asarray(b_hh2, np.float32)[2 * DH:].reshape(1, DH))
    shared["onesb"] = _bf16(np.ones((1, BS)))
    if cfg["has_out_b"]:
        shared["outb"] = _f8(np.asarray(out_b).reshape(1, -1),
                             PSCALE * HSCALE)
        shared["ones"] = _f8(np.ones((1, 128)), 1.0)

    words = emb[np.asarray(target_variable[:, :TD], dtype=np.int64)]  # [B,TD,DW]

    in_maps = []
    for c in range(NC):
        sl = slice(c * BS, (c + 1) * BS)
        vs = vid_feats[sl]                      # [BS, NF, DV]
        ws = words[sl]                          # [BS, TD, DW]
        m = dict(shared)
        m["xT"] = _f8e4(vs.transpose(2, 1, 0).reshape(cfg["DV"], -1), 8.0)
        m["wordsT"] = _bf16(ws.transpose(2, 1, 0).reshape(cfg["DW"], -1))
        in_maps.append(m)
    return in_maps


_CACHE = {}
LAST_RESULT = None


def kernel(**inputs):
    global LAST_RESULT
    from concourse.bass_utils import run_bass_kernel_spmd

    DH = 512
    has_out_b = bool(np.any(np.asarray(inputs["out_b"])))
    c1 = np.asarray(inputs["b_ih1"], np.float32).copy()
    c1[:2 * DH] += np.asarray(inputs["b_hh1"], np.float32)[:2 * DH]
    has_bias1 = bool(np.any(c1))
    has_bhn1 = bool(np.any(np.asarray(inputs["b_hh1"])[2 * DH:]))
    has_bhn2 = bool(np.any(np.asarray(inputs["b_hh2"])[2 * DH:]))
    key = ("full", has_out_b, has_bias1, has_bhn1, has_bhn2)
    if key not in _CACHE:
        cfg = make_cfg(has_out_b=has_out_b, has_bias1=has_bias1,
                       has_bhn1=has_bhn1, has_bhn2=has_bhn2)
        _CACHE[key] = (cfg, build_nc(cfg))
    cfg, nc = _CACHE[key]

    in_maps = prep_inputs(cfg, **inputs)
    res = run_bass_kernel_spmd(nc, in_maps, list(range(cfg["n_cores"])))
    LAST_RESULT = res
    outs = [np.asarray(res.results[c]["out"]) for c in range(cfg["n_cores"])]
    return np.concatenate(outs, axis=0).astype(np.float32)  # [B, TD, V]
